# revision 25
# baseline (speedup 1.0000x reference)
"""Trainium2 Bass kernel for nn_HeteroForecastSageConv (v4).

Strategy (8 NeuronCores, SPMD, dst-sharded):
 - Each core owns 12800 target nodes. Inputs are host-rotated per core
   so the own shard is always columns [0:12800) of xT; the
   pretransformed tables (x_t, x_c) are computed replicated with the
   stationary-operand matmul trick (lhsT = x^T tile, rhs = W) yielding
   node-major PSUM directly (no transposes). Per-type bias is folded
   into the input on the host (x' = x + b @ W^-1; zero here).
 - Phase A window also runs the whole ct pipeline: Pool does ct
   gathers, DVE builds ct one-hots between relu work, PE accumulates
   aggC into persistent SBUF. xt_mine (feature-major own shard) is
   computed from the same streamed chunks.
 - Phase B: tt gathers (Pool) + fused one-hot + matmul aggregation
   feature-major, then the folded epilogue. The one-hot mean-scale is
   fused: DVE tensor_scalar (-iota == -dloc) * inv; a fraction of
   one-hots is offloaded to the Activation engine via the exact
   identity inv*relu(1 - (i-d)^2) built with Square+Relu activations.
 - Engine split: x reads + tt idx + output on SP; table writes
   Act/Pool; relu 2/3 DVE 1/3 Act; gathers Pool only.

Math (alpha = 0.5, folded on host):
  x_mid = x_t @ (0.5 W_self + 0.5 W_ct_r + I) + aggS @ (0.25 W_s2d)
        + aggD @ (0.25 W_d2s) + aggC @ (0.5 W_ct_l) + b_mid
  out   = relu(x_mid) @ W_out + b_out
"""
import sys
import dataclasses

sys.path.insert(0, "/opt/trn_rl_repo")

import numpy as np
import ml_dtypes

import concourse.bass as bass
import concourse.bacc as bacc
import concourse.mybir as mybir
import concourse.tile as tile
from concourse import bass_utils

BF16 = ml_dtypes.bfloat16
F32 = np.float32
NCORE = 8
P = 128


@dataclasses.dataclass(frozen=True)
class Cfg:
    n_t: int
    n_c: int
    shard: int
    nc_pad: int
    nbin: int
    grp: int
    act_oh_every: int = 10 ** 9   # every Nth tt one-hot goes to the Act engine

    @property
    def nt_pad(self):
        return self.shard * NCORE

    @property
    def nblk(self):
        return self.shard // P

    @property
    def bins(self):
        # 2048-aligned uneven bins, each <= 32767 for int16 gather indices
        assert self.nbin == 4 and self.nt_pad == 102400
        return [0, 26624, 53248, 79872, 102400]


FULL = Cfg(n_t=100000, n_c=20000, shard=12800, nc_pad=20480, nbin=4, grp=4)


def _perm_rows(r):
    """Table-row permutation: within each 2048-row chunk, node (128g + p)
    is stored at row (16p + g) so phase-A writes are 4KB-contiguous."""
    chunk = r // 2048
    o = r % 2048
    return chunk * 2048 + (o % P) * 16 + o // P

_prog_cache = {}


def _groups(cfg):
    return [(g0, min(cfg.grp, cfg.nblk - g0)) for g0 in range(0, cfg.nblk, cfg.grp)]


def _wrap_idx(stream):
    assert stream.size % 16 == 0
    idx16 = stream.reshape(-1, 16).T
    return np.ascontiguousarray(np.tile(idx16, (8, 1)).astype(np.int16))


def build_program(cfg: Cfg, cc_tt, cc_ct):
    """cc_tt: [nblk, nbin, 2] tiles per tt cell; cc_ct: [nblk] (shared cores)."""
    dt = mybir.dt
    AF = mybir.ActivationFunctionType
    OP = mybir.AluOpType
    nblk, nbin = cfg.nblk, cfg.nbin
    groups = _groups(cfg)

    tt_bin_len = [0] * nbin
    tt_call_off = {}
    tt_call_len = {}
    for gi, (g0, gn) in enumerate(groups):
        for d in range(2):
            for b in range(nbin):
                tt_call_off[(gi, d, b)] = tt_bin_len[b]
                L = int(sum(cc_tt[g0 + j, b, d] for j in range(gn))) * P
                tt_call_len[(gi, d, b)] = L
                tt_bin_len[b] += L
    ct_len = 0
    ct_call_off = {}
    ct_call_len = {}
    for gi, (g0, gn) in enumerate(groups):
        ct_call_off[gi] = ct_len
        L = int(sum(cc_ct[g0 + j] for j in range(gn))) * P
        ct_call_len[gi] = L
        ct_len += L
    tt_pieces = int(cc_tt.sum())
    ct_pieces = int(cc_ct.sum())

    nc = bacc.Bacc("TRN2", target_bir_lowering=False, debug=False)

    def din(name, shape, d):
        return nc.dram_tensor(name, shape, d, kind="ExternalInput")

    t_xT = din("xT", [P, cfg.nt_pad], dt.bfloat16)
    t_xcT = din("xcT", [P, cfg.nc_pad], dt.bfloat16)
    t_wpt = din("wpt", [P, P], dt.bfloat16)
    t_wpc = din("wpc", [P, P], dt.bfloat16)
    t_bpt = din("bpt", [P, 1], dt.float32)
    t_w1 = din("w1", [P, P], dt.bfloat16)
    t_ws = din("ws", [P, P], dt.bfloat16)
    t_wd = din("wd", [P, P], dt.bfloat16)
    t_wc = din("wc", [P, P], dt.bfloat16)
    t_wo = din("wo", [P, P], dt.bfloat16)
    t_bmid = din("bmid", [P, 1], dt.float32)
    t_bout = din("bout", [P, 1], dt.float32)
    t_iota = din("iota", [P, P], dt.bfloat16)       # +i, for Act Square path
    t_iotan = din("iotan", [P, P], dt.bfloat16)     # -i, for DVE is_equal path
    t_idx = [din(f"idx{b}", [P, max(tt_bin_len[b], 16) // 16], dt.int16)
             for b in range(nbin)]
    t_idxc = din("idxc", [P, max(ct_len, 16) // 16], dt.int16)
    t_dlsn = din("dlsn", [P, tt_pieces], dt.float32)    # -dloc (pad +1)
    t_inv = din("invv", [P, tt_pieces], dt.float32)     # +inv
    t_invn = din("invvn", [P, tt_pieces], dt.float32)   # -inv
    t_dlsnc = din("dlsnc", [P, ct_pieces], dt.float32)
    t_invc = din("invvc", [P, ct_pieces], dt.float32)
    t_out = nc.dram_tensor("outT", [P, cfg.shard], dt.bfloat16, kind="ExternalOutput")

    with tile.TileContext(nc) as tc:
        with tc.tile_pool(name="dram", bufs=1, space="DRAM") as dpool, \
             tc.tile_pool(name="persist", bufs=1) as pp:
            xtn = dpool.tile([cfg.nt_pad, P], dt.bfloat16)
            xcn = dpool.tile([cfg.nc_pad, P], dt.bfloat16)

            def load(t, shape, d, eng=None):
                s = pp.tile(shape, d, name=f"sb_{t.name}")
                (eng or nc.sync).dma_start(s[:], t.ap())
                return s

            sb_wpt = load(t_wpt, [P, P], dt.bfloat16)
            sb_wpc = load(t_wpc, [P, P], dt.bfloat16)
            sb_bpt = load(t_bpt, [P, 1], dt.float32)
            sb_w1 = load(t_w1, [P, P], dt.bfloat16)
            sb_ws = load(t_ws, [P, P], dt.bfloat16)
            sb_wd = load(t_wd, [P, P], dt.bfloat16)
            sb_wc = load(t_wc, [P, P], dt.bfloat16)
            sb_wo = load(t_wo, [P, P], dt.bfloat16)
            sb_bmid = load(t_bmid, [P, 1], dt.float32)
            sb_bout = load(t_bout, [P, 1], dt.float32)
            sb_iota = load(t_iota, [P, P], dt.bfloat16)
            sb_iotan = load(t_iotan, [P, P], dt.bfloat16)
            # ct streams on Pool (idle at start); tt streams on SP
            sb_idxc = load(t_idxc, [P, max(ct_len, 16) // 16], dt.int16,
                           eng=nc.gpsimd)
            sb_dlsnc = load(t_dlsnc, [P, ct_pieces], dt.float32, eng=nc.gpsimd)
            sb_invc = load(t_invc, [P, ct_pieces], dt.float32, eng=nc.gpsimd)
            sb_idx = [load(t_idx[b], [P, max(tt_bin_len[b], 16) // 16],
                           dt.int16, eng=nc.gpsimd) for b in range(nbin)]
            sb_dlsn = load(t_dlsn, [P, tt_pieces], dt.float32, eng=nc.gpsimd)
            sb_inv = load(t_inv, [P, tt_pieces], dt.float32, eng=nc.gpsimd)
            sb_invn = load(t_invn, [P, tt_pieces], dt.float32, eng=nc.gpsimd)
            xt_mine = pp.tile([P, cfg.shard], dt.bfloat16)
            aggC = pp.tile([P, cfg.shard], dt.bfloat16)

            # =============== Phase A + ct pipeline (interleaved) ===============
            CHUNK = 2048
            relu_tick = [0]
            ctp_ctr = [0]
            wr_tick = [0]

            with tc.tile_pool(name="pa", bufs=3) as pa, \
                 tc.tile_pool(name="pgc", bufs=3) as pgc, \
                 tc.tile_pool(name="psA", bufs=4, space="PSUM") as psA, \
                 tc.tile_pool(name="psAm", bufs=2, space="PSUM") as psAm, \
                 tc.tile_pool(name="psC", bufs=2, space="PSUM") as psC:

                def emit_chunk(src_dram, w_sb, nodes_dram, c0, cw, mine):
                    sb_in = pa.tile([P, CHUNK], dt.bfloat16, name="a_in", tag="a_in")
                    r_eng = nc.scalar if (c0 // CHUNK) % 4 == 3 else nc.sync
                    r_eng.dma_start(sb_in[:, :cw], src_dram.ap()[:, c0:c0 + cw])
                    sb_nodes = pa.tile([P, CHUNK // P, P], dt.bfloat16,
                                       name="a_nodes", tag="a_nodes")
                    for s0 in range(0, cw, 512):
                        ps = psA.tile([P, 4, P], dt.float32, name="a_ps", tag="a_ps")
                        for j in range(4):
                            nc.tensor.matmul(ps[:, j, :],
                                             lhsT=sb_in[:, s0 + P * j:s0 + P * (j + 1)],
                                             rhs=w_sb[:], start=True, stop=True)
                        dst = sb_nodes[:, s0 // P:s0 // P + 4, :]
                        if relu_tick[0] % 9 < 5:
                            nc.vector.tensor_scalar(
                                out=dst, in0=ps[:], scalar1=0.0, scalar2=None,
                                op0=OP.max)
                        else:
                            nc.scalar.activation(dst, ps[:], AF.Relu)
                        relu_tick[0] += 1
                        if mine and c0 + s0 < cfg.shard:
                            psm = psAm.tile([P, 512], dt.float32, name="m_ps",
                                            tag="m_ps")
                            nc.tensor.matmul(psm[:], lhsT=w_sb[:],
                                             rhs=sb_in[:, s0:s0 + 512],
                                             start=True, stop=True)
                            if relu_tick[0] % 9 < 5:
                                nc.vector.tensor_scalar(
                                    out=xt_mine[:, c0 + s0:c0 + s0 + 512],
                                    in0=psm[:], scalar1=sb_bpt[:, 0:1], scalar2=0.0,
                                    op0=OP.add, op1=OP.max)
                            else:
                                nc.scalar.activation(
                                    xt_mine[:, c0 + s0:c0 + s0 + 512], psm[:],
                                    AF.Relu, bias=sb_bpt[:, 0:1])
                            relu_tick[0] += 1
                    w_eng = (nc.sync, nc.scalar, nc.sync)[wr_tick[0] % 3]
                    wr_tick[0] += 1
                    w_eng.dma_start(
                        nodes_dram[c0:c0 + cw, :].rearrange("(p g) f -> p g f", p=P),
                        sb_nodes[:, :cw // P, :])

                for c0 in range(0, cfg.nc_pad, CHUNK):
                    emit_chunk(t_xcT, sb_wpc, xcn, c0,
                               min(CHUNK, cfg.nc_pad - c0), False)

                # ct gathers, one per group (throttled by pgc pool)
                xgc_tiles = []
                for gi, (g0, gn) in enumerate(groups):
                    Lc = ct_call_len[gi]
                    t = pgc.tile([P, Lc // P, P], dt.bfloat16, name="xgc", tag="xgc")
                    nc.gpsimd.dma_gather(
                        out_ap=t[:], in_ap=xcn[:],
                        idxs_ap=sb_idxc[:, ct_call_off[gi] // 16:
                                        (ct_call_off[gi] + Lc) // 16],
                        num_idxs=Lc, num_idxs_reg=Lc,
                        elem_size=P, single_packet=False)
                    xgc_tiles.append(t)

                def ct_units():
                    for gi, (g0, gn) in enumerate(groups):
                        for p0 in range(0, gn, 2):
                            yield (gi, g0, p0, min(2, gn - p0))

                def emit_ct_unit(gi, g0, p0, pn):
                    xgc = xgc_tiles[gi]
                    ps_c = psC.tile([P, pn * P], dt.float32, name="cps", tag="cps")
                    for bl in range(pn):
                        j = p0 + bl
                        blk = g0 + j
                        basec = sum(int(cc_ct[g0 + jj]) for jj in range(j))
                        n_mmc = int(cc_ct[blk])
                        for c in range(n_mmc):
                            oh_t = pa.tile([P, P], dt.bfloat16, name="ohc", tag="ohc")
                            col = ctp_ctr[0]
                            nc.vector.tensor_scalar(
                                out=oh_t[:], in0=sb_iotan[:],
                                scalar1=sb_dlsnc[:, col:col + 1],
                                scalar2=sb_invc[:, col:col + 1],
                                op0=OP.is_equal, op1=OP.mult)
                            ctp_ctr[0] += 1
                            nc.tensor.matmul(
                                ps_c[:, bl * P:(bl + 1) * P],
                                lhsT=xgc[:, basec + c, :], rhs=oh_t[:],
                                start=(c == 0), stop=(c == n_mmc - 1))
                    blk0 = g0 + p0
                    nc.scalar.activation(aggC[:, blk0 * P:(blk0 + pn) * P],
                                         ps_c[:], AF.Copy)

                ct_it = iter(ct_units())
                for i, c0 in enumerate(range(0, cfg.nt_pad, CHUNK)):
                    emit_chunk(t_xT, sb_wpt, xtn, c0,
                               min(CHUNK, cfg.nt_pad - c0), True)
                    if i >= 2:
                        u = next(ct_it, None)
                        if u is not None:
                            emit_ct_unit(*u)
                for u in ct_it:
                    emit_ct_unit(*u)

            # ======================= Phase B: tt + epilogue =======================
            piece_ctr = [0]
            oh_tick = [0]

            with tc.tile_pool(name="pb", bufs=2) as pb, \
                 tc.tile_pool(name="oh", bufs=8) as poh, \
                 tc.tile_pool(name="ohsq", bufs=4) as psq, \
                 tc.tile_pool(name="psAgg", bufs=2, space="PSUM") as psG, \
                 tc.tile_pool(name="psMid", bufs=2, space="PSUM") as psM, \
                 tc.tile_pool(name="psOut", bufs=2, space="PSUM") as psO:

                def make_oh_tt():
                    col = piece_ctr[0]
                    piece_ctr[0] += 1
                    oh_t = poh.tile([P, P], dt.bfloat16, name="oh", tag="oh")
                    if oh_tick[0] % cfg.act_oh_every == cfg.act_oh_every - 1:
                        sq = psq.tile([P, P], dt.bfloat16, name="sq", tag="sq")
                        nc.scalar.activation(sq[:], sb_iota[:], AF.Square,
                                             bias=sb_dlsn[:, col:col + 1])
                        nc.scalar.activation(oh_t[:], sq[:], AF.Relu,
                                             scale=sb_invn[:, col:col + 1],
                                             bias=sb_inv[:, col:col + 1])
                    else:
                        nc.vector.tensor_scalar(
                            out=oh_t[:], in0=sb_iotan[:],
                            scalar1=sb_dlsn[:, col:col + 1],
                            scalar2=sb_inv[:, col:col + 1],
                            op0=OP.is_equal, op1=OP.mult)
                    oh_tick[0] += 1
                    return oh_t

                for gi, (g0, gn) in enumerate(groups):
                    xg = {}
                    for d in range(2):
                        for b in range(nbin):
                            L = tt_call_len[(gi, d, b)]
                            t = pb.tile([P, L // P, P], dt.bfloat16,
                                        name=f"xg{d}{b}", tag=f"xg{d}{b}")
                            nc.gpsimd.dma_gather(
                                out_ap=t[:],
                                in_ap=xtn[cfg.bins[b]:cfg.bins[b + 1], :],
                                idxs_ap=sb_idx[b][:, tt_call_off[(gi, d, b)] // 16:
                                                  (tt_call_off[(gi, d, b)] + L) // 16],
                                num_idxs=L, num_idxs_reg=L,
                                elem_size=P, single_packet=False)
                            xg[(d, b)] = t

                    agg_sb = {}
                    for p0 in range(0, gn, 2):
                        pn = min(2, gn - p0)
                        ps_agg = psG.tile([P, pn * 256], dt.float32,
                                          name="agg", tag="agg")
                        for bl in range(pn):
                            j = p0 + bl
                            blk = g0 + j
                            for d in range(2):
                                k = 0
                                n_mm = int(sum(cc_tt[blk, b, d] for b in range(nbin)))
                                for b in range(nbin):
                                    base = sum(int(cc_tt[g0 + jj, b, d])
                                               for jj in range(j))
                                    for c in range(int(cc_tt[blk, b, d])):
                                        oh_t = make_oh_tt()
                                        nc.tensor.matmul(
                                            ps_agg[:, bl * 256 + d * P:
                                                   bl * 256 + (d + 1) * P],
                                            lhsT=xg[(d, b)][:, base + c, :],
                                            rhs=oh_t[:],
                                            start=(k == 0), stop=(k == n_mm - 1))
                                        k += 1
                        sb_a = pb.tile([P, pn * 256], dt.bfloat16, name="aggsb",
                                       tag="aggsb")
                        nc.scalar.activation(sb_a[:], ps_agg[:], AF.Copy)
                        for bl in range(pn):
                            agg_sb[p0 + bl] = (sb_a, bl * 256)

                    og = pb.tile([P, gn * P], dt.bfloat16, name="og", tag="og")
                    for h0 in range(0, gn, 4):
                        hn = min(4, gn - h0)
                        ps_mid = psM.tile([P, hn * P], dt.float32, name="mid",
                                          tag="mid")
                        for bl in range(hn):
                            j = h0 + bl
                            blk = g0 + j
                            sb_a, off = agg_sb[j]
                            reg = ps_mid[:, bl * P:bl * P + P]
                            nc.tensor.matmul(reg, lhsT=sb_w1[:],
                                             rhs=xt_mine[:, P * blk:P * blk + P],
                                             start=True, stop=False)
                            nc.tensor.matmul(reg, lhsT=sb_ws[:],
                                             rhs=sb_a[:, off:off + P],
                                             start=False, stop=False)
                            nc.tensor.matmul(reg, lhsT=sb_wd[:],
                                             rhs=sb_a[:, off + P:off + 2 * P],
                                             start=False, stop=False)
                            nc.tensor.matmul(reg, lhsT=sb_wc[:],
                                             rhs=aggC[:, P * blk:P * blk + P],
                                             start=False, stop=True)
                        sb_mid = pb.tile([P, hn * P], dt.bfloat16, name="mid_sb",
                                         tag="mid_sb")
                        nc.scalar.activation(sb_mid[:], ps_mid[:], AF.Relu,
                                             bias=sb_bmid[:, 0:1])
                        ps_out = psO.tile([P, hn * P], dt.float32, name="out_ps",
                                          tag="out_ps")
                        for bl in range(hn):
                            nc.tensor.matmul(ps_out[:, bl * P:bl * P + P],
                                             lhsT=sb_wo[:],
                                             rhs=sb_mid[:, bl * P:bl * P + P],
                                             start=True, stop=True)
                        nc.scalar.activation(og[:, h0 * P:(h0 + hn) * P], ps_out[:],
                                             AF.Identity, bias=sb_bout[:, 0:1])
                    nc.sync.dma_start(t_out.ap()[:, P * g0:P * (g0 + gn)],
                                      og[:, :P * gn])

    nc.compile()
    return nc


def preprocess(inputs, cfg: Cfg):
    xt = np.asarray(inputs["x_target"], F32)
    xc = np.asarray(inputs["x_context"], F32)
    ett = np.asarray(inputs["edge_tt"]).astype(np.int64)
    ecs = np.asarray(inputs["edge_ct_src"]).astype(np.int64)
    ecd = np.asarray(inputs["edge_ct_dst"]).astype(np.int64)
    nblk, nbin = cfg.nblk, cfg.nbin
    groups = _groups(cfg)

    def fold_bias(x, W, b):
        if not np.any(b):
            return x
        c = np.linalg.lstsq(np.asarray(W, np.float64).T,
                            np.asarray(b, np.float64), rcond=None)[0]
        return x + c[None, :].astype(F32)

    Wp_t = np.asarray(inputs["Wp_t"], F32)
    Wp_c = np.asarray(inputs["Wp_c"], F32)
    bp_t = np.asarray(inputs["bp_t"], F32)
    bp_c = np.asarray(inputs["bp_c"], F32)
    xtf = fold_bias(xt, Wp_t, bp_t)
    xcf = fold_bias(xc, Wp_c, bp_c)

    xtT = np.zeros((P, cfg.nt_pad), BF16)
    xtT[:, :xt.shape[0]] = xtf.T.astype(BF16)
    xcT = np.zeros((P, cfg.nc_pad), BF16)
    xcT[:, :xc.shape[0]] = xcf.T.astype(BF16)

    W_self = np.asarray(inputs["W_self"], F32)
    W_ct_r = np.asarray(inputs["W_ct_r"], F32)
    w1 = 0.5 * W_self + 0.5 * W_ct_r + np.eye(P, dtype=F32)
    ws = 0.25 * np.asarray(inputs["W_s2d"], F32)
    wd = 0.25 * np.asarray(inputs["W_d2s"], F32)
    wc = 0.5 * np.asarray(inputs["W_ct_l"], F32)
    wo = np.asarray(inputs["W_out"], F32)
    bmid = (0.5 * np.asarray(inputs["b_self"], F32)
            + 0.25 * np.asarray(inputs["b_s2d"], F32)
            + 0.25 * np.asarray(inputs["b_d2s"], F32)
            + 0.5 * np.asarray(inputs["b_ct_l"], F32))
    bout = np.asarray(inputs["b_out"], F32)

    iota = np.arange(P, dtype=F32)
    shared = {
        "xcT": xcT,
        "wpt": np.ascontiguousarray(Wp_t.astype(BF16)),
        "wpc": np.ascontiguousarray(Wp_c.astype(BF16)),
        "bpt": bp_t.reshape(P, 1),
        "w1": w1.astype(BF16), "ws": ws.astype(BF16), "wd": wd.astype(BF16),
        "wc": wc.astype(BF16), "wo": wo.astype(BF16),
        "bmid": bmid.reshape(P, 1), "bout": bout.reshape(P, 1),
        "iota": np.ascontiguousarray(np.broadcast_to(iota, (P, P)).astype(BF16)),
        "iotan": np.ascontiguousarray(np.broadcast_to(-iota, (P, P)).astype(BF16)),
    }

    # per-core rotated source locations
    dirs = [
        ("s", ett[1], ett[0], True),
        ("d", ett[0], ett[1], True),
        ("c", ecd, ecs, False),
    ]

    pre = {}
    for nm, key, gnode, is_tt in dirs:
        core = (key // cfg.shard).astype(np.int64)
        blk = ((key % cfg.shard) // P).astype(np.int64)
        dloc = (key % P).astype(F32)
        cnt = np.bincount(key, minlength=cfg.nt_pad)
        inv = (1.0 / np.maximum(cnt, 1)).astype(F32)
        invv = inv[key]
        if is_tt:
            rot = (gnode - core * cfg.shard) % cfg.nt_pad
            r2 = _perm_rows(rot)
            bins = np.asarray(cfg.bins)
            bin_ = np.searchsorted(bins, r2, side="right") - 1
            loc = (r2 - bins[bin_]).astype(np.int16)
            cell = ((core * nblk + blk) * nbin + bin_)
            ncell = NCORE * nblk * nbin
        else:
            loc = _perm_rows(gnode).astype(np.int16)
            cell = core * nblk + blk
            ncell = NCORE * nblk
        order = np.argsort(cell, kind="stable")
        counts = np.bincount(cell, minlength=ncell)
        pre[nm] = dict(order=order, cell_s=cell[order], counts=counts,
                       loc=loc, dloc=dloc, invv=invv)

    cnt_s = pre["s"]["counts"].reshape(NCORE, nblk, nbin)
    cnt_d = pre["d"]["counts"].reshape(NCORE, nblk, nbin)
    cnt_c = pre["c"]["counts"].reshape(NCORE, nblk)
    cc_tt = np.zeros((nblk, nbin, 2), np.int64)
    cc_tt[:, :, 0] = np.maximum(-(-cnt_s.max(axis=0) // P), 1)
    cc_tt[:, :, 1] = np.maximum(-(-cnt_d.max(axis=0) // P), 1)
    cc_ct = np.maximum(-(-cnt_c.max(axis=0) // P), 1)

    def blk_offs(cc):
        if cc.ndim == 2:
            offs = np.zeros((nblk, nbin), np.int64)
            for b in range(nbin):
                o = 0
                for gi, (g0, gn) in enumerate(groups):
                    for j in range(gn):
                        offs[g0 + j, b] = o
                        o += int(cc[g0 + j, b]) * P
            return offs
        offs = np.zeros(nblk, np.int64)
        o = 0
        for gi, (g0, gn) in enumerate(groups):
            for j in range(gn):
                offs[g0 + j] = o
                o += int(cc[g0 + j]) * P
        return offs

    offs_s = blk_offs(cc_tt[:, :, 0])
    offs_d = blk_offs(cc_tt[:, :, 1])
    offs_c = blk_offs(cc_ct)

    def fill(nm, cc, offs):
        d = pre[nm]
        order, cell_s, counts = d["order"], d["cell_s"], d["counts"]
        starts = np.concatenate([[0], np.cumsum(counts)[:-1]])
        loc_s = d["loc"][order]
        dloc_s = d["dloc"][order]
        invv_s = d["invv"][order]
        is_tt = cc.ndim == 2
        cap = cc * P
        if is_tt:
            bin_len = [int(cap[:, b].sum()) for b in range(nbin)]
        else:
            bin_len = [int(cap.sum())]
        pos_in_cell = np.arange(len(cell_s)) - starts[cell_s]
        if is_tt:
            core_of = cell_s // (nblk * nbin)
            blk_of = (cell_s // nbin) % nblk
            bin_of = cell_s % nbin
            binbase = np.concatenate([[0], np.cumsum(bin_len)])[:-1]
            gslot = binbase[bin_of] + offs[blk_of, bin_of] + pos_in_cell
        else:
            core_of = cell_s // nblk
            blk_of = cell_s % nblk
            gslot = offs[blk_of] + pos_in_cell
        total = int(sum(bin_len))
        idx = np.zeros((NCORE, total), np.int16)
        dls = np.full((NCORE, total), -1.0, F32)
        inv = np.zeros((NCORE, total), F32)
        idx[core_of, gslot] = loc_s
        dls[core_of, gslot] = dloc_s
        inv[core_of, gslot] = invv_s
        if is_tt:
            binbase = np.concatenate([[0], np.cumsum(bin_len)])
            return ([idx[:, binbase[b]:binbase[b + 1]] for b in range(nbin)],
                    [dls[:, binbase[b]:binbase[b + 1]] for b in range(nbin)],
                    [inv[:, binbase[b]:binbase[b + 1]] for b in range(nbin)])
        return [idx], [dls], [inv]

    idx_s, dls_s, inv_s = fill("s", cc_tt[:, :, 0], offs_s)
    idx_d, dls_d, inv_d = fill("d", cc_tt[:, :, 1], offs_d)
    idx_c, dls_c, inv_c = fill("c", cc_ct, offs_c)

    def tt_piece_stream(core):
        cols_d, cols_i = [], []
        for gi, (g0, gn) in enumerate(groups):
            for j in range(gn):
                blk = g0 + j
                for d in range(2):
                    dls_bins = dls_s if d == 0 else dls_d
                    inv_bins = inv_s if d == 0 else inv_d
                    offs = offs_s if d == 0 else offs_d
                    ccd = cc_tt[:, :, d]
                    for b in range(nbin):
                        o = int(offs[blk, b])
                        for c in range(int(ccd[blk, b])):
                            cols_d.append(dls_bins[b][core, o + c * P:o + (c + 1) * P])
                            cols_i.append(inv_bins[b][core, o + c * P:o + (c + 1) * P])
        return (np.ascontiguousarray(np.stack(cols_d, axis=1)),
                np.ascontiguousarray(np.stack(cols_i, axis=1)))

    def ct_piece_stream(core):
        cols_d, cols_i = [], []
        for gi, (g0, gn) in enumerate(groups):
            for j in range(gn):
                blk = g0 + j
                o = int(offs_c[blk])
                for c in range(int(cc_ct[blk])):
                    cols_d.append(dls_c[0][core, o + c * P:o + (c + 1) * P])
                    cols_i.append(inv_c[0][core, o + c * P:o + (c + 1) * P])
        return (np.ascontiguousarray(np.stack(cols_d, axis=1)),
                np.ascontiguousarray(np.stack(cols_i, axis=1)))

    in_maps = []
    for k in range(NCORE):
        m = dict(shared)
        m["xT"] = np.roll(xtT, -cfg.shard * k, axis=1)
        for b in range(nbin):
            segs = []
            for gi, (g0, gn) in enumerate(groups):
                for d in range(2):
                    src = idx_s[b] if d == 0 else idx_d[b]
                    offs = offs_s if d == 0 else offs_d
                    o = int(offs[g0, b])
                    L = int(sum(cc_tt[g0 + j, b, d] for j in range(gn))) * P
                    segs.append(src[k, o:o + L])
            m[f"idx{b}"] = _wrap_idx(np.concatenate(segs))
        m["idxc"] = _wrap_idx(idx_c[0][k])
        dls_t, inv_t = tt_piece_stream(k)
        m["dlsn"] = np.ascontiguousarray(-dls_t)
        m["invv"] = inv_t
        m["invvn"] = np.ascontiguousarray(-inv_t)
        dls_ct, inv_ct = ct_piece_stream(k)
        m["dlsnc"] = np.ascontiguousarray(-dls_ct)
        m["invvc"] = inv_ct
        in_maps.append(m)
    return in_maps, cc_tt, cc_ct


def run(inputs, cfg: Cfg, trace=False):
    in_maps, cc_tt, cc_ct = preprocess(inputs, cfg)
    key = (cfg, cc_tt.tobytes(), cc_ct.tobytes())
    if key not in _prog_cache:
        _prog_cache[key] = build_program(cfg, cc_tt, cc_ct)
    nc = _prog_cache[key]
    res = bass_utils.run_bass_kernel_spmd(nc, in_maps, core_ids=list(range(NCORE)),
                                          trace=trace)
    outT = np.concatenate([res.results[k]["outT"] for k in range(NCORE)], axis=1)
    n_t = np.asarray(inputs["x_target"]).shape[0]
    out = outT[:, :n_t].T.astype(F32)
    return out, res


def kernel(**inputs) -> np.ndarray:
    out, _ = run(inputs, FULL, trace=False)
    return out


# revision 32
# speedup vs baseline: 1.1372x; 1.1372x over previous
"""Trainium2 Bass kernel for nn_HeteroForecastSageConv (v4).

Strategy (8 NeuronCores, SPMD, dst-sharded):
 - Each core owns 12800 target nodes. Inputs are host-rotated per core
   so the own shard is always columns [0:12800) of xT; the
   pretransformed tables (x_t, x_c) are computed replicated with the
   stationary-operand matmul trick (lhsT = x^T tile, rhs = W) yielding
   node-major PSUM directly (no transposes). Per-type bias is folded
   into the input on the host (x' = x + b @ W^-1; zero here).
 - Phase A window also runs the whole ct pipeline: Pool does ct
   gathers, DVE builds ct one-hots between relu work, PE accumulates
   aggC into persistent SBUF. xt_mine (feature-major own shard) is
   computed from the same streamed chunks.
 - Phase B: tt gathers (Pool) + fused one-hot + matmul aggregation
   feature-major, then the folded epilogue. The one-hot mean-scale is
   fused: DVE tensor_scalar (-iota == -dloc) * inv; a fraction of
   one-hots is offloaded to the Activation engine via the exact
   identity inv*relu(1 - (i-d)^2) built with Square+Relu activations.
 - Engine split: x reads + tt idx + output on SP; table writes
   Act/Pool; relu 2/3 DVE 1/3 Act; gathers Pool only.

Math (alpha = 0.5, folded on host):
  x_mid = x_t @ (0.5 W_self + 0.5 W_ct_r + I) + aggS @ (0.25 W_s2d)
        + aggD @ (0.25 W_d2s) + aggC @ (0.5 W_ct_l) + b_mid
  out   = relu(x_mid) @ W_out + b_out
"""
import sys
import dataclasses

sys.path.insert(0, "/opt/trn_rl_repo")

import numpy as np
import ml_dtypes

import concourse.bass as bass
import concourse.bacc as bacc
import concourse.mybir as mybir
import concourse.tile as tile
from concourse import bass_utils

BF16 = ml_dtypes.bfloat16
F32 = np.float32
NCORE = 8
P = 128


@dataclasses.dataclass(frozen=True)
class Cfg:
    n_t: int
    n_c: int
    shard: int
    nc_pad: int
    nbin: int
    grp: int
    act_oh_every: int = 10 ** 9   # every Nth tt one-hot goes to the Act engine

    @property
    def nt_pad(self):
        return self.shard * NCORE

    @property
    def nblk(self):
        return self.shard // P

    @property
    def bins(self):
        # 2048-aligned uneven bins, each <= 32767 for int16 gather indices
        assert self.nbin == 4 and self.nt_pad == 102400
        return [0, 26624, 53248, 79872, 102400]


FULL = Cfg(n_t=100000, n_c=20000, shard=12800, nc_pad=20480, nbin=4, grp=4)


def _perm_rows(r):
    """Table-row permutation: within each 2048-row chunk, node (128g + p)
    is stored at row (16p + g) so phase-A writes are 4KB-contiguous."""
    chunk = r // 2048
    o = r % 2048
    return chunk * 2048 + (o % P) * 16 + o // P

_prog_cache = {}


def _groups(cfg):
    return [(g0, min(cfg.grp, cfg.nblk - g0)) for g0 in range(0, cfg.nblk, cfg.grp)]


def _wrap_idx(stream):
    assert stream.size % 16 == 0
    idx16 = stream.reshape(-1, 16).T
    return np.ascontiguousarray(np.tile(idx16, (8, 1)).astype(np.int16))


def build_program(cfg: Cfg, cc_tt, cc_ct):
    """cc_tt: [nblk, nbin, 2] tiles per tt cell; cc_ct: [nblk] (shared cores)."""
    dt = mybir.dt
    AF = mybir.ActivationFunctionType
    OP = mybir.AluOpType
    nblk, nbin = cfg.nblk, cfg.nbin
    groups = _groups(cfg)

    tt_bin_len = [0] * nbin
    tt_call_off = {}
    tt_call_len = {}
    for gi, (g0, gn) in enumerate(groups):
        for d in range(2):
            for b in range(nbin):
                tt_call_off[(gi, d, b)] = tt_bin_len[b]
                L = int(sum(cc_tt[g0 + j, b, d] for j in range(gn))) * P
                tt_call_len[(gi, d, b)] = L
                tt_bin_len[b] += L
    ct_len = 0
    ct_call_off = {}
    ct_call_len = {}
    for gi, (g0, gn) in enumerate(groups):
        ct_call_off[gi] = ct_len
        L = int(sum(cc_ct[g0 + j] for j in range(gn))) * P
        ct_call_len[gi] = L
        ct_len += L
    tt_pieces = int(cc_tt.sum())
    ct_pieces = int(cc_ct.sum())

    nc = bacc.Bacc("TRN2", target_bir_lowering=False, debug=False)

    def din(name, shape, d):
        return nc.dram_tensor(name, shape, d, kind="ExternalInput")

    t_xT = din("xT", [P, cfg.nt_pad], dt.bfloat16)
    t_xcT = din("xcT", [P, cfg.nc_pad], dt.bfloat16)
    t_wpt = din("wpt", [P, P], dt.bfloat16)
    t_wpc = din("wpc", [P, P], dt.bfloat16)
    t_bpt = din("bpt", [P, 1], dt.float32)
    t_w1 = din("w1", [P, P], dt.bfloat16)
    t_ws = din("ws", [P, P], dt.bfloat16)
    t_wd = din("wd", [P, P], dt.bfloat16)
    t_wc = din("wc", [P, P], dt.bfloat16)
    t_wo = din("wo", [P, P], dt.bfloat16)
    t_bmid = din("bmid", [P, 1], dt.float32)
    t_bout = din("bout", [P, 1], dt.float32)
    t_iota = din("iota", [P, P], dt.bfloat16)       # +i, for Act Square path
    t_iotan = din("iotan", [P, P], dt.bfloat16)     # -i, for DVE is_equal path
    t_idx = [din(f"idx{b}", [P, max(tt_bin_len[b], 16) // 16], dt.int16)
             for b in range(nbin)]
    t_idxc = din("idxc", [P, max(ct_len, 16) // 16], dt.int16)
    t_dlsn = din("dlsn", [P, tt_pieces], dt.float32)    # -dloc (pad +1)
    t_inv = din("invv", [P, tt_pieces], dt.float32)     # +inv
    t_invn = din("invvn", [P, tt_pieces], dt.float32)   # -inv
    t_dlsnc = din("dlsnc", [P, ct_pieces], dt.float32)
    t_invc = din("invvc", [P, ct_pieces], dt.float32)
    t_out = nc.dram_tensor("outT", [P, cfg.shard], dt.bfloat16, kind="ExternalOutput")

    with tile.TileContext(nc) as tc:
        with tc.tile_pool(name="dram", bufs=1, space="DRAM") as dpool, \
             tc.tile_pool(name="persist", bufs=1) as pp:
            xtn = dpool.tile([cfg.nt_pad, P], dt.bfloat16)
            xcn = dpool.tile([cfg.nc_pad, P], dt.bfloat16)

            def load(t, shape, d, eng=None):
                s = pp.tile(shape, d, name=f"sb_{t.name}")
                (eng or nc.sync).dma_start(s[:], t.ap())
                return s

            sb_wpt = load(t_wpt, [P, P], dt.bfloat16)
            sb_wpc = load(t_wpc, [P, P], dt.bfloat16)
            sb_bpt = load(t_bpt, [P, 1], dt.float32)
            sb_w1 = load(t_w1, [P, P], dt.bfloat16)
            sb_ws = load(t_ws, [P, P], dt.bfloat16)
            sb_wd = load(t_wd, [P, P], dt.bfloat16)
            sb_wc = load(t_wc, [P, P], dt.bfloat16)
            sb_wo = load(t_wo, [P, P], dt.bfloat16)
            sb_bmid = load(t_bmid, [P, 1], dt.float32)
            sb_bout = load(t_bout, [P, 1], dt.float32)
            sb_iota = load(t_iota, [P, P], dt.bfloat16)
            sb_iotan = load(t_iotan, [P, P], dt.bfloat16)
            # ct streams on Pool (idle at start); tt streams on SP
            sb_idxc = load(t_idxc, [P, max(ct_len, 16) // 16], dt.int16,
                           eng=nc.gpsimd)
            sb_dlsnc = load(t_dlsnc, [P, ct_pieces], dt.float32, eng=nc.gpsimd)
            sb_invc = load(t_invc, [P, ct_pieces], dt.float32, eng=nc.gpsimd)
            sb_idx = [load(t_idx[b], [P, max(tt_bin_len[b], 16) // 16],
                           dt.int16, eng=nc.gpsimd) for b in range(nbin)]
            sb_dlsn = load(t_dlsn, [P, tt_pieces], dt.float32, eng=nc.gpsimd)
            sb_inv = load(t_inv, [P, tt_pieces], dt.float32, eng=nc.gpsimd)
            sb_invn = load(t_invn, [P, tt_pieces], dt.float32, eng=nc.gpsimd)
            xt_mine = pp.tile([P, cfg.shard], dt.bfloat16)
            aggC = pp.tile([P, cfg.shard], dt.bfloat16)

            # =============== Phase A + ct pipeline (interleaved) ===============
            CHUNK = 2048
            relu_tick = [0]
            ctp_ctr = [0]
            wr_tick = [0]

            with tc.tile_pool(name="pa", bufs=3) as pa, \
                 tc.tile_pool(name="pgc", bufs=3) as pgc, \
                 tc.tile_pool(name="psA", bufs=4, space="PSUM") as psA, \
                 tc.tile_pool(name="psAm", bufs=2, space="PSUM") as psAm, \
                 tc.tile_pool(name="psC", bufs=2, space="PSUM") as psC:

                def emit_chunk(src_dram, w_sb, nodes_dram, c0, cw, mine):
                    sb_in = pa.tile([P, CHUNK], dt.bfloat16, name="a_in", tag="a_in")
                    r_eng = nc.scalar if (c0 // CHUNK) % 4 == 3 else nc.sync
                    r_eng.dma_start(sb_in[:, :cw], src_dram.ap()[:, c0:c0 + cw])
                    sb_nodes = pa.tile([P, CHUNK // P, P], dt.bfloat16,
                                       name="a_nodes", tag="a_nodes")
                    for s0 in range(0, cw, 512):
                        ps = psA.tile([P, 4, P], dt.float32, name="a_ps", tag="a_ps")
                        for j in range(4):
                            nc.tensor.matmul(ps[:, j, :],
                                             lhsT=sb_in[:, s0 + P * j:s0 + P * (j + 1)],
                                             rhs=w_sb[:], start=True, stop=True)
                        dst = sb_nodes[:, s0 // P:s0 // P + 4, :]
                        if relu_tick[0] % 9 < 5:
                            nc.vector.tensor_scalar(
                                out=dst, in0=ps[:], scalar1=0.0, scalar2=None,
                                op0=OP.max)
                        else:
                            nc.scalar.activation(dst, ps[:], AF.Relu)
                        relu_tick[0] += 1
                        if mine and c0 + s0 < cfg.shard:
                            psm = psAm.tile([P, 512], dt.float32, name="m_ps",
                                            tag="m_ps")
                            nc.tensor.matmul(psm[:], lhsT=w_sb[:],
                                             rhs=sb_in[:, s0:s0 + 512],
                                             start=True, stop=True)
                            if relu_tick[0] % 9 < 5:
                                nc.vector.tensor_scalar(
                                    out=xt_mine[:, c0 + s0:c0 + s0 + 512],
                                    in0=psm[:], scalar1=sb_bpt[:, 0:1], scalar2=0.0,
                                    op0=OP.add, op1=OP.max)
                            else:
                                nc.scalar.activation(
                                    xt_mine[:, c0 + s0:c0 + s0 + 512], psm[:],
                                    AF.Relu, bias=sb_bpt[:, 0:1])
                            relu_tick[0] += 1
                    w_eng = (nc.sync, nc.scalar, nc.sync)[wr_tick[0] % 3]
                    wr_tick[0] += 1
                    w_eng.dma_start(
                        nodes_dram[c0:c0 + cw, :].rearrange("(p g) f -> p g f", p=P),
                        sb_nodes[:, :cw // P, :])

                for c0 in range(0, cfg.nc_pad, CHUNK):
                    emit_chunk(t_xcT, sb_wpc, xcn, c0,
                               min(CHUNK, cfg.nc_pad - c0), False)

                # ct gathers, one per group (throttled by pgc pool)
                xgc_tiles = []
                for gi, (g0, gn) in enumerate(groups):
                    Lc = ct_call_len[gi]
                    t = pgc.tile([P, Lc // P, P], dt.bfloat16, name="xgc", tag="xgc")
                    nc.gpsimd.dma_gather(
                        out_ap=t[:], in_ap=xcn[:],
                        idxs_ap=sb_idxc[:, ct_call_off[gi] // 16:
                                        (ct_call_off[gi] + Lc) // 16],
                        num_idxs=Lc, num_idxs_reg=Lc,
                        elem_size=P, single_packet=False)
                    xgc_tiles.append(t)

                def ct_units():
                    for gi, (g0, gn) in enumerate(groups):
                        for p0 in range(0, gn, 2):
                            yield (gi, g0, p0, min(2, gn - p0))

                def emit_ct_unit(gi, g0, p0, pn):
                    xgc = xgc_tiles[gi]
                    ps_c = psC.tile([P, pn * P], dt.float32, name="cps", tag="cps")
                    for bl in range(pn):
                        j = p0 + bl
                        blk = g0 + j
                        basec = sum(int(cc_ct[g0 + jj]) for jj in range(j))
                        n_mmc = int(cc_ct[blk])
                        for c in range(n_mmc):
                            oh_t = pa.tile([P, P], dt.bfloat16, name="ohc", tag="ohc")
                            col = ctp_ctr[0]
                            nc.vector.tensor_scalar(
                                out=oh_t[:], in0=sb_iotan[:],
                                scalar1=sb_dlsnc[:, col:col + 1],
                                scalar2=sb_invc[:, col:col + 1],
                                op0=OP.is_equal, op1=OP.mult)
                            ctp_ctr[0] += 1
                            nc.tensor.matmul(
                                ps_c[:, bl * P:(bl + 1) * P],
                                lhsT=xgc[:, basec + c, :], rhs=oh_t[:],
                                start=(c == 0), stop=(c == n_mmc - 1))
                    blk0 = g0 + p0
                    nc.scalar.activation(aggC[:, blk0 * P:(blk0 + pn) * P],
                                         ps_c[:], AF.Copy)

                ct_it = iter(ct_units())
                for i, c0 in enumerate(range(0, cfg.nt_pad, CHUNK)):
                    emit_chunk(t_xT, sb_wpt, xtn, c0,
                               min(CHUNK, cfg.nt_pad - c0), True)
                    if i >= 2:
                        u = next(ct_it, None)
                        if u is not None:
                            emit_ct_unit(*u)
                for u in ct_it:
                    emit_ct_unit(*u)

            # ======================= Phase B: tt + epilogue =======================
            piece_ctr = [0]
            oh_tick = [0]

            with tc.tile_pool(name="pb", bufs=2) as pb, \
                 tc.tile_pool(name="pxg", bufs=3) as pxg, \
                 tc.tile_pool(name="oh", bufs=16) as poh, \
                 tc.tile_pool(name="ohsq", bufs=4) as psq, \
                 tc.tile_pool(name="psAgg", bufs=2, space="PSUM") as psG, \
                 tc.tile_pool(name="psMid", bufs=2, space="PSUM") as psM, \
                 tc.tile_pool(name="psOut", bufs=2, space="PSUM") as psO:

                def make_oh_tt():
                    col = piece_ctr[0]
                    piece_ctr[0] += 1
                    oh_t = poh.tile([P, P], dt.bfloat16, name="oh", tag="oh")
                    if oh_tick[0] % cfg.act_oh_every == cfg.act_oh_every - 1:
                        sq = psq.tile([P, P], dt.bfloat16, name="sq", tag="sq")
                        nc.scalar.activation(sq[:], sb_iota[:], AF.Square,
                                             bias=sb_dlsn[:, col:col + 1])
                        nc.scalar.activation(oh_t[:], sq[:], AF.Relu,
                                             scale=sb_invn[:, col:col + 1],
                                             bias=sb_inv[:, col:col + 1])
                    else:
                        nc.vector.tensor_scalar(
                            out=oh_t[:], in0=sb_iotan[:],
                            scalar1=sb_dlsn[:, col:col + 1],
                            scalar2=sb_inv[:, col:col + 1],
                            op0=OP.is_equal, op1=OP.mult)
                    oh_tick[0] += 1
                    return oh_t

                for gi, (g0, gn) in enumerate(groups):
                    xg = {}
                    for d in range(2):
                        for b in range(nbin):
                            L = tt_call_len[(gi, d, b)]
                            t = pxg.tile([P, L // P, P], dt.bfloat16,
                                         name=f"xg{d}{b}", tag=f"xg{d}{b}")
                            nc.gpsimd.dma_gather(
                                out_ap=t[:],
                                in_ap=xtn[cfg.bins[b]:cfg.bins[b + 1], :],
                                idxs_ap=sb_idx[b][:, tt_call_off[(gi, d, b)] // 16:
                                                  (tt_call_off[(gi, d, b)] + L) // 16],
                                num_idxs=L, num_idxs_reg=L,
                                elem_size=P, single_packet=False)
                            xg[(d, b)] = t

                    agg_sb = {}
                    for p0 in range(0, gn, 2):
                        pn = min(2, gn - p0)
                        ps_agg = psG.tile([P, pn * 256], dt.float32,
                                          name="agg", tag="agg")
                        for bl in range(pn):
                            j = p0 + bl
                            blk = g0 + j
                            for d in range(2):
                                k = 0
                                n_mm = int(sum(cc_tt[blk, b, d] for b in range(nbin)))
                                for b in range(nbin):
                                    base = sum(int(cc_tt[g0 + jj, b, d])
                                               for jj in range(j))
                                    for c in range(int(cc_tt[blk, b, d])):
                                        oh_t = make_oh_tt()
                                        nc.tensor.matmul(
                                            ps_agg[:, bl * 256 + d * P:
                                                   bl * 256 + (d + 1) * P],
                                            lhsT=xg[(d, b)][:, base + c, :],
                                            rhs=oh_t[:],
                                            start=(k == 0), stop=(k == n_mm - 1))
                                        k += 1
                        sb_a = pb.tile([P, pn * 256], dt.bfloat16, name="aggsb",
                                       tag="aggsb")
                        nc.scalar.activation(sb_a[:], ps_agg[:], AF.Copy)
                        for bl in range(pn):
                            agg_sb[p0 + bl] = (sb_a, bl * 256)

                    og = pb.tile([P, gn * P], dt.bfloat16, name="og", tag="og")
                    for h0 in range(0, gn, 4):
                        hn = min(4, gn - h0)
                        ps_mid = psM.tile([P, hn * P], dt.float32, name="mid",
                                          tag="mid")
                        for bl in range(hn):
                            j = h0 + bl
                            blk = g0 + j
                            sb_a, off = agg_sb[j]
                            reg = ps_mid[:, bl * P:bl * P + P]
                            nc.tensor.matmul(reg, lhsT=sb_w1[:],
                                             rhs=xt_mine[:, P * blk:P * blk + P],
                                             start=True, stop=False)
                            nc.tensor.matmul(reg, lhsT=sb_ws[:],
                                             rhs=sb_a[:, off:off + P],
                                             start=False, stop=False)
                            nc.tensor.matmul(reg, lhsT=sb_wd[:],
                                             rhs=sb_a[:, off + P:off + 2 * P],
                                             start=False, stop=False)
                            nc.tensor.matmul(reg, lhsT=sb_wc[:],
                                             rhs=aggC[:, P * blk:P * blk + P],
                                             start=False, stop=True)
                        sb_mid = pb.tile([P, hn * P], dt.bfloat16, name="mid_sb",
                                         tag="mid_sb")
                        nc.scalar.activation(sb_mid[:], ps_mid[:], AF.Relu,
                                             bias=sb_bmid[:, 0:1])
                        ps_out = psO.tile([P, hn * P], dt.float32, name="out_ps",
                                          tag="out_ps")
                        for bl in range(hn):
                            nc.tensor.matmul(ps_out[:, bl * P:bl * P + P],
                                             lhsT=sb_wo[:],
                                             rhs=sb_mid[:, bl * P:bl * P + P],
                                             start=True, stop=True)
                        nc.scalar.activation(og[:, h0 * P:(h0 + hn) * P], ps_out[:],
                                             AF.Identity, bias=sb_bout[:, 0:1])
                    nc.sync.dma_start(t_out.ap()[:, P * g0:P * (g0 + gn)],
                                      og[:, :P * gn])

    nc.compile()
    return nc


def preprocess(inputs, cfg: Cfg):
    xt = np.asarray(inputs["x_target"], F32)
    xc = np.asarray(inputs["x_context"], F32)
    ett = np.asarray(inputs["edge_tt"]).astype(np.int64)
    ecs = np.asarray(inputs["edge_ct_src"]).astype(np.int64)
    ecd = np.asarray(inputs["edge_ct_dst"]).astype(np.int64)
    nblk, nbin = cfg.nblk, cfg.nbin
    groups = _groups(cfg)

    def fold_bias(x, W, b):
        if not np.any(b):
            return x
        c = np.linalg.lstsq(np.asarray(W, np.float64).T,
                            np.asarray(b, np.float64), rcond=None)[0]
        return x + c[None, :].astype(F32)

    Wp_t = np.asarray(inputs["Wp_t"], F32)
    Wp_c = np.asarray(inputs["Wp_c"], F32)
    bp_t = np.asarray(inputs["bp_t"], F32)
    bp_c = np.asarray(inputs["bp_c"], F32)
    xtf = fold_bias(xt, Wp_t, bp_t)
    xcf = fold_bias(xc, Wp_c, bp_c)

    xtT = np.zeros((P, cfg.nt_pad), BF16)
    xtT[:, :xt.shape[0]] = xtf.T.astype(BF16)
    xcT = np.zeros((P, cfg.nc_pad), BF16)
    xcT[:, :xc.shape[0]] = xcf.T.astype(BF16)

    W_self = np.asarray(inputs["W_self"], F32)
    W_ct_r = np.asarray(inputs["W_ct_r"], F32)
    w1 = 0.5 * W_self + 0.5 * W_ct_r + np.eye(P, dtype=F32)
    ws = 0.25 * np.asarray(inputs["W_s2d"], F32)
    wd = 0.25 * np.asarray(inputs["W_d2s"], F32)
    wc = 0.5 * np.asarray(inputs["W_ct_l"], F32)
    wo = np.asarray(inputs["W_out"], F32)
    bmid = (0.5 * np.asarray(inputs["b_self"], F32)
            + 0.25 * np.asarray(inputs["b_s2d"], F32)
            + 0.25 * np.asarray(inputs["b_d2s"], F32)
            + 0.5 * np.asarray(inputs["b_ct_l"], F32))
    bout = np.asarray(inputs["b_out"], F32)

    iota = np.arange(P, dtype=F32)
    shared = {
        "xcT": xcT,
        "wpt": np.ascontiguousarray(Wp_t.astype(BF16)),
        "wpc": np.ascontiguousarray(Wp_c.astype(BF16)),
        "bpt": bp_t.reshape(P, 1),
        "w1": w1.astype(BF16), "ws": ws.astype(BF16), "wd": wd.astype(BF16),
        "wc": wc.astype(BF16), "wo": wo.astype(BF16),
        "bmid": bmid.reshape(P, 1), "bout": bout.reshape(P, 1),
        "iota": np.ascontiguousarray(np.broadcast_to(iota, (P, P)).astype(BF16)),
        "iotan": np.ascontiguousarray(np.broadcast_to(-iota, (P, P)).astype(BF16)),
    }

    # per-core rotated source locations
    dirs = [
        ("s", ett[1], ett[0], True),
        ("d", ett[0], ett[1], True),
        ("c", ecd, ecs, False),
    ]

    pre = {}
    for nm, key, gnode, is_tt in dirs:
        core = (key // cfg.shard).astype(np.int64)
        blk = ((key % cfg.shard) // P).astype(np.int64)
        dloc = (key % P).astype(F32)
        cnt = np.bincount(key, minlength=cfg.nt_pad)
        inv = (1.0 / np.maximum(cnt, 1)).astype(F32)
        invv = inv[key]
        if is_tt:
            rot = (gnode - core * cfg.shard) % cfg.nt_pad
            r2 = _perm_rows(rot)
            bins = np.asarray(cfg.bins)
            bin_ = np.searchsorted(bins, r2, side="right") - 1
            loc = (r2 - bins[bin_]).astype(np.int16)
            cell = ((core * nblk + blk) * nbin + bin_)
            ncell = NCORE * nblk * nbin
        else:
            loc = _perm_rows(gnode).astype(np.int16)
            cell = core * nblk + blk
            ncell = NCORE * nblk
        order = np.argsort(cell, kind="stable")
        counts = np.bincount(cell, minlength=ncell)
        pre[nm] = dict(order=order, cell_s=cell[order], counts=counts,
                       loc=loc, dloc=dloc, invv=invv)

    cnt_s = pre["s"]["counts"].reshape(NCORE, nblk, nbin)
    cnt_d = pre["d"]["counts"].reshape(NCORE, nblk, nbin)
    cnt_c = pre["c"]["counts"].reshape(NCORE, nblk)
    cc_tt = np.zeros((nblk, nbin, 2), np.int64)
    cc_tt[:, :, 0] = np.maximum(-(-cnt_s.max(axis=0) // P), 1)
    cc_tt[:, :, 1] = np.maximum(-(-cnt_d.max(axis=0) // P), 1)
    cc_ct = np.maximum(-(-cnt_c.max(axis=0) // P), 1)

    def blk_offs(cc):
        if cc.ndim == 2:
            offs = np.zeros((nblk, nbin), np.int64)
            for b in range(nbin):
                o = 0
                for gi, (g0, gn) in enumerate(groups):
                    for j in range(gn):
                        offs[g0 + j, b] = o
                        o += int(cc[g0 + j, b]) * P
            return offs
        offs = np.zeros(nblk, np.int64)
        o = 0
        for gi, (g0, gn) in enumerate(groups):
            for j in range(gn):
                offs[g0 + j] = o
                o += int(cc[g0 + j]) * P
        return offs

    offs_s = blk_offs(cc_tt[:, :, 0])
    offs_d = blk_offs(cc_tt[:, :, 1])
    offs_c = blk_offs(cc_ct)

    def fill(nm, cc, offs):
        d = pre[nm]
        order, cell_s, counts = d["order"], d["cell_s"], d["counts"]
        starts = np.concatenate([[0], np.cumsum(counts)[:-1]])
        loc_s = d["loc"][order]
        dloc_s = d["dloc"][order]
        invv_s = d["invv"][order]
        is_tt = cc.ndim == 2
        cap = cc * P
        if is_tt:
            bin_len = [int(cap[:, b].sum()) for b in range(nbin)]
        else:
            bin_len = [int(cap.sum())]
        pos_in_cell = np.arange(len(cell_s)) - starts[cell_s]
        if is_tt:
            core_of = cell_s // (nblk * nbin)
            blk_of = (cell_s // nbin) % nblk
            bin_of = cell_s % nbin
            binbase = np.concatenate([[0], np.cumsum(bin_len)])[:-1]
            gslot = binbase[bin_of] + offs[blk_of, bin_of] + pos_in_cell
        else:
            core_of = cell_s // nblk
            blk_of = cell_s % nblk
            gslot = offs[blk_of] + pos_in_cell
        total = int(sum(bin_len))
        idx = np.zeros((NCORE, total), np.int16)
        dls = np.full((NCORE, total), -1.0, F32)
        inv = np.zeros((NCORE, total), F32)
        idx[core_of, gslot] = loc_s
        dls[core_of, gslot] = dloc_s
        inv[core_of, gslot] = invv_s
        if is_tt:
            binbase = np.concatenate([[0], np.cumsum(bin_len)])
            return ([idx[:, binbase[b]:binbase[b + 1]] for b in range(nbin)],
                    [dls[:, binbase[b]:binbase[b + 1]] for b in range(nbin)],
                    [inv[:, binbase[b]:binbase[b + 1]] for b in range(nbin)])
        return [idx], [dls], [inv]

    idx_s, dls_s, inv_s = fill("s", cc_tt[:, :, 0], offs_s)
    idx_d, dls_d, inv_d = fill("d", cc_tt[:, :, 1], offs_d)
    idx_c, dls_c, inv_c = fill("c", cc_ct, offs_c)

    def tt_piece_stream(core):
        cols_d, cols_i = [], []
        for gi, (g0, gn) in enumerate(groups):
            for j in range(gn):
                blk = g0 + j
                for d in range(2):
                    dls_bins = dls_s if d == 0 else dls_d
                    inv_bins = inv_s if d == 0 else inv_d
                    offs = offs_s if d == 0 else offs_d
                    ccd = cc_tt[:, :, d]
                    for b in range(nbin):
                        o = int(offs[blk, b])
                        for c in range(int(ccd[blk, b])):
                            cols_d.append(dls_bins[b][core, o + c * P:o + (c + 1) * P])
                            cols_i.append(inv_bins[b][core, o + c * P:o + (c + 1) * P])
        return (np.ascontiguousarray(np.stack(cols_d, axis=1)),
                np.ascontiguousarray(np.stack(cols_i, axis=1)))

    def ct_piece_stream(core):
        cols_d, cols_i = [], []
        for gi, (g0, gn) in enumerate(groups):
            for j in range(gn):
                blk = g0 + j
                o = int(offs_c[blk])
                for c in range(int(cc_ct[blk])):
                    cols_d.append(dls_c[0][core, o + c * P:o + (c + 1) * P])
                    cols_i.append(inv_c[0][core, o + c * P:o + (c + 1) * P])
        return (np.ascontiguousarray(np.stack(cols_d, axis=1)),
                np.ascontiguousarray(np.stack(cols_i, axis=1)))

    in_maps = []
    for k in range(NCORE):
        m = dict(shared)
        m["xT"] = np.roll(xtT, -cfg.shard * k, axis=1)
        for b in range(nbin):
            segs = []
            for gi, (g0, gn) in enumerate(groups):
                for d in range(2):
                    src = idx_s[b] if d == 0 else idx_d[b]
                    offs = offs_s if d == 0 else offs_d
                    o = int(offs[g0, b])
                    L = int(sum(cc_tt[g0 + j, b, d] for j in range(gn))) * P
                    segs.append(src[k, o:o + L])
            m[f"idx{b}"] = _wrap_idx(np.concatenate(segs))
        m["idxc"] = _wrap_idx(idx_c[0][k])
        dls_t, inv_t = tt_piece_stream(k)
        m["dlsn"] = np.ascontiguousarray(-dls_t)
        m["invv"] = inv_t
        m["invvn"] = np.ascontiguousarray(-inv_t)
        dls_ct, inv_ct = ct_piece_stream(k)
        m["dlsnc"] = np.ascontiguousarray(-dls_ct)
        m["invvc"] = inv_ct
        in_maps.append(m)
    return in_maps, cc_tt, cc_ct


def run(inputs, cfg: Cfg, trace=False):
    in_maps, cc_tt, cc_ct = preprocess(inputs, cfg)
    key = (cfg, cc_tt.tobytes(), cc_ct.tobytes())
    if key not in _prog_cache:
        _prog_cache[key] = build_program(cfg, cc_tt, cc_ct)
    nc = _prog_cache[key]
    res = bass_utils.run_bass_kernel_spmd(nc, in_maps, core_ids=list(range(NCORE)),
                                          trace=trace)
    outT = np.concatenate([res.results[k]["outT"] for k in range(NCORE)], axis=1)
    n_t = np.asarray(inputs["x_target"]).shape[0]
    out = outT[:, :n_t].T.astype(F32)
    return out, res


def kernel(**inputs) -> np.ndarray:
    out, _ = run(inputs, FULL, trace=False)
    return out


# revision 35
# speedup vs baseline: 1.1653x; 1.0248x over previous
"""Trainium2 Bass kernel for nn_HeteroForecastSageConv (v4).

Strategy (8 NeuronCores, SPMD, dst-sharded):
 - Each core owns 12800 target nodes. Inputs are host-rotated per core
   so the own shard is always columns [0:12800) of xT; the
   pretransformed tables (x_t, x_c) are computed replicated with the
   stationary-operand matmul trick (lhsT = x^T tile, rhs = W) yielding
   node-major PSUM directly (no transposes). Per-type bias is folded
   into the input on the host (x' = x + b @ W^-1; zero here).
 - Phase A window also runs the whole ct pipeline: Pool does ct
   gathers, DVE builds ct one-hots between relu work, PE accumulates
   aggC into persistent SBUF. xt_mine (feature-major own shard) is
   computed from the same streamed chunks.
 - Phase B: tt gathers (Pool) + fused one-hot + matmul aggregation
   feature-major, then the folded epilogue. The one-hot mean-scale is
   fused: DVE tensor_scalar (-iota == -dloc) * inv. Table rows are
   chunk-permuted so phase-A writes are 4KB-contiguous per partition.
 - Engine split: x reads + most table writes + output on SP; 1/3
   writes + some reads on Act; relu 5/9 DVE 4/9 Act; gathers + idx
   stream loads on Pool only.

Math (alpha = 0.5, folded on host):
  x_mid = x_t @ (0.5 W_self + 0.5 W_ct_r + I) + aggS @ (0.25 W_s2d)
        + aggD @ (0.25 W_d2s) + aggC @ (0.5 W_ct_l) + b_mid
  out   = relu(x_mid) @ W_out + b_out
"""
import sys
import dataclasses

sys.path.insert(0, "/opt/trn_rl_repo")

import numpy as np
import ml_dtypes

import concourse.bass as bass
import concourse.bacc as bacc
import concourse.mybir as mybir
import concourse.tile as tile
from concourse import bass_utils

BF16 = ml_dtypes.bfloat16
F32 = np.float32
NCORE = 8
P = 128


@dataclasses.dataclass(frozen=True)
class Cfg:
    n_t: int
    n_c: int
    shard: int
    nc_pad: int
    nbin: int
    grp: int
    act_oh_every: int = 10 ** 9   # every Nth tt one-hot goes to the Act engine

    @property
    def nt_pad(self):
        return self.shard * NCORE

    @property
    def nblk(self):
        return self.shard // P

    @property
    def bins(self):
        # 2048-aligned uneven bins, each <= 32767 for int16 gather indices
        assert self.nbin == 4 and self.nt_pad == 102400
        return [0, 26624, 53248, 79872, 102400]


FULL = Cfg(n_t=100000, n_c=20000, shard=12800, nc_pad=20480, nbin=4, grp=4)


def _perm_rows(r):
    """Table-row permutation: within each 2048-row chunk, node (128g + p)
    is stored at row (16p + g) so phase-A writes are 4KB-contiguous."""
    chunk = r // 2048
    o = r % 2048
    return chunk * 2048 + (o % P) * 16 + o // P

_prog_cache = {}


def _groups(cfg):
    return [(g0, min(cfg.grp, cfg.nblk - g0)) for g0 in range(0, cfg.nblk, cfg.grp)]


def _wrap_idx(stream):
    assert stream.size % 16 == 0
    idx16 = stream.reshape(-1, 16).T
    return np.ascontiguousarray(np.tile(idx16, (8, 1)).astype(np.int16))


def build_program(cfg: Cfg, cc_tt, cc_ct):
    """cc_tt: [nblk, nbin, 2] tiles per tt cell; cc_ct: [nblk] (shared cores)."""
    dt = mybir.dt
    AF = mybir.ActivationFunctionType
    OP = mybir.AluOpType
    nblk, nbin = cfg.nblk, cfg.nbin
    groups = _groups(cfg)

    tt_bin_len = [0] * nbin
    tt_call_off = {}
    tt_call_len = {}
    for gi, (g0, gn) in enumerate(groups):
        for d in range(2):
            for b in range(nbin):
                tt_call_off[(gi, d, b)] = tt_bin_len[b]
                L = int(sum(cc_tt[g0 + j, b, d] for j in range(gn))) * P
                tt_call_len[(gi, d, b)] = L
                tt_bin_len[b] += L
    ct_len = 0
    ct_call_off = {}
    ct_call_len = {}
    for gi, (g0, gn) in enumerate(groups):
        ct_call_off[gi] = ct_len
        L = int(sum(cc_ct[g0 + j] for j in range(gn))) * P
        ct_call_len[gi] = L
        ct_len += L
    tt_pieces = int(cc_tt.sum())
    ct_pieces = int(cc_ct.sum())

    nc = bacc.Bacc("TRN2", target_bir_lowering=False, debug=False)

    def din(name, shape, d):
        return nc.dram_tensor(name, shape, d, kind="ExternalInput")

    t_xT = din("xT", [P, cfg.nt_pad], dt.bfloat16)
    t_xcT = din("xcT", [P, cfg.nc_pad], dt.bfloat16)
    t_wpt = din("wpt", [P, P], dt.bfloat16)
    t_wpc = din("wpc", [P, P], dt.bfloat16)
    t_bpt = din("bpt", [P, 1], dt.float32)
    t_w1 = din("w1", [P, P], dt.bfloat16)
    t_ws = din("ws", [P, P], dt.bfloat16)
    t_wd = din("wd", [P, P], dt.bfloat16)
    t_wc = din("wc", [P, P], dt.bfloat16)
    t_wo = din("wo", [P, P], dt.bfloat16)
    t_bmid = din("bmid", [P, 1], dt.float32)
    t_bout = din("bout", [P, 1], dt.float32)
    t_iota = din("iota", [P, P], dt.bfloat16)       # +i, for Act Square path
    t_iotan = din("iotan", [P, P], dt.bfloat16)     # -i, for DVE is_equal path
    t_idx = [din(f"idx{b}", [P, max(tt_bin_len[b], 16) // 16], dt.int16)
             for b in range(nbin)]
    t_idxc = din("idxc", [P, max(ct_len, 16) // 16], dt.int16)
    t_dlsn = din("dlsn", [P, tt_pieces], dt.float32)    # -dloc (pad +1)
    t_inv = din("invv", [P, tt_pieces], dt.float32)     # +inv
    t_invn = din("invvn", [P, tt_pieces], dt.float32)   # -inv
    t_dlsnc = din("dlsnc", [P, ct_pieces], dt.float32)
    t_invc = din("invvc", [P, ct_pieces], dt.float32)
    t_out = nc.dram_tensor("outT", [P, cfg.shard], dt.bfloat16, kind="ExternalOutput")

    with tile.TileContext(nc) as tc:
        with tc.tile_pool(name="dram", bufs=1, space="DRAM") as dpool, \
             tc.tile_pool(name="persist", bufs=1) as pp:
            xtn = dpool.tile([cfg.nt_pad, P], dt.bfloat16)
            xcn = dpool.tile([cfg.nc_pad, P], dt.bfloat16)

            def load(t, shape, d, eng=None):
                s = pp.tile(shape, d, name=f"sb_{t.name}")
                (eng or nc.sync).dma_start(s[:], t.ap())
                return s

            sb_wpt = load(t_wpt, [P, P], dt.bfloat16)
            sb_wpc = load(t_wpc, [P, P], dt.bfloat16)
            sb_bpt = load(t_bpt, [P, 1], dt.float32)
            sb_w1 = load(t_w1, [P, P], dt.bfloat16)
            sb_ws = load(t_ws, [P, P], dt.bfloat16)
            sb_wd = load(t_wd, [P, P], dt.bfloat16)
            sb_wc = load(t_wc, [P, P], dt.bfloat16)
            sb_wo = load(t_wo, [P, P], dt.bfloat16)
            sb_bmid = load(t_bmid, [P, 1], dt.float32)
            sb_bout = load(t_bout, [P, 1], dt.float32)
            sb_iota = load(t_iota, [P, P], dt.bfloat16)
            sb_iotan = load(t_iotan, [P, P], dt.bfloat16)
            # ct streams on Pool (idle at start); tt streams on SP
            sb_idxc = load(t_idxc, [P, max(ct_len, 16) // 16], dt.int16,
                           eng=nc.gpsimd)
            sb_dlsnc = load(t_dlsnc, [P, ct_pieces], dt.float32, eng=nc.gpsimd)
            sb_invc = load(t_invc, [P, ct_pieces], dt.float32, eng=nc.gpsimd)
            sb_idx = [load(t_idx[b], [P, max(tt_bin_len[b], 16) // 16],
                           dt.int16, eng=nc.gpsimd) for b in range(nbin)]
            sb_dlsn = load(t_dlsn, [P, tt_pieces], dt.float32, eng=nc.gpsimd)
            sb_inv = load(t_inv, [P, tt_pieces], dt.float32, eng=nc.gpsimd)
            sb_invn = load(t_invn, [P, tt_pieces], dt.float32, eng=nc.gpsimd)
            xt_mine = pp.tile([P, cfg.shard], dt.bfloat16)
            aggC = pp.tile([P, cfg.shard], dt.bfloat16)

            # =============== Phase A + ct pipeline (interleaved) ===============
            CHUNK = 2048
            relu_tick = [0]
            ctp_ctr = [0]
            wr_tick = [0]

            with tc.tile_pool(name="pa", bufs=4) as pa, \
                 tc.tile_pool(name="pgc", bufs=4) as pgc, \
                 tc.tile_pool(name="psA", bufs=4, space="PSUM") as psA, \
                 tc.tile_pool(name="psAm", bufs=2, space="PSUM") as psAm, \
                 tc.tile_pool(name="psC", bufs=2, space="PSUM") as psC:

                def emit_chunk(src_dram, w_sb, nodes_dram, c0, cw, mine):
                    sb_in = pa.tile([P, CHUNK], dt.bfloat16, name="a_in", tag="a_in")
                    r_eng = nc.scalar if (c0 // CHUNK) % 4 == 3 else nc.sync
                    r_eng.dma_start(sb_in[:, :cw], src_dram.ap()[:, c0:c0 + cw])
                    sb_nodes = pa.tile([P, CHUNK // P, P], dt.bfloat16,
                                       name="a_nodes", tag="a_nodes")
                    for s0 in range(0, cw, 512):
                        ps = psA.tile([P, 4, P], dt.float32, name="a_ps", tag="a_ps")
                        for j in range(4):
                            nc.tensor.matmul(ps[:, j, :],
                                             lhsT=sb_in[:, s0 + P * j:s0 + P * (j + 1)],
                                             rhs=w_sb[:], start=True, stop=True)
                        dst = sb_nodes[:, s0 // P:s0 // P + 4, :]
                        if relu_tick[0] % 9 < 5:
                            nc.vector.tensor_scalar(
                                out=dst, in0=ps[:], scalar1=0.0, scalar2=None,
                                op0=OP.max)
                        else:
                            nc.scalar.activation(dst, ps[:], AF.Relu)
                        relu_tick[0] += 1
                        if mine and c0 + s0 < cfg.shard:
                            psm = psAm.tile([P, 512], dt.float32, name="m_ps",
                                            tag="m_ps")
                            nc.tensor.matmul(psm[:], lhsT=w_sb[:],
                                             rhs=sb_in[:, s0:s0 + 512],
                                             start=True, stop=True)
                            if relu_tick[0] % 9 < 5:
                                nc.vector.tensor_scalar(
                                    out=xt_mine[:, c0 + s0:c0 + s0 + 512],
                                    in0=psm[:], scalar1=sb_bpt[:, 0:1], scalar2=0.0,
                                    op0=OP.add, op1=OP.max)
                            else:
                                nc.scalar.activation(
                                    xt_mine[:, c0 + s0:c0 + s0 + 512], psm[:],
                                    AF.Relu, bias=sb_bpt[:, 0:1])
                            relu_tick[0] += 1
                    w_eng = (nc.sync, nc.scalar, nc.sync)[wr_tick[0] % 3]
                    wr_tick[0] += 1
                    w_eng.dma_start(
                        nodes_dram[c0:c0 + cw, :].rearrange("(p g) f -> p g f", p=P),
                        sb_nodes[:, :cw // P, :])

                for c0 in range(0, cfg.nc_pad, CHUNK):
                    emit_chunk(t_xcT, sb_wpc, xcn, c0,
                               min(CHUNK, cfg.nc_pad - c0), False)

                # ct gathers, one per group (throttled by pgc pool)
                xgc_tiles = []
                for gi, (g0, gn) in enumerate(groups):
                    Lc = ct_call_len[gi]
                    t = pgc.tile([P, Lc // P, P], dt.bfloat16, name="xgc", tag="xgc")
                    nc.gpsimd.dma_gather(
                        out_ap=t[:], in_ap=xcn[:],
                        idxs_ap=sb_idxc[:, ct_call_off[gi] // 16:
                                        (ct_call_off[gi] + Lc) // 16],
                        num_idxs=Lc, num_idxs_reg=Lc,
                        elem_size=P, single_packet=False)
                    xgc_tiles.append(t)

                def ct_units():
                    for gi, (g0, gn) in enumerate(groups):
                        for p0 in range(0, gn, 2):
                            yield (gi, g0, p0, min(2, gn - p0))

                def emit_ct_unit(gi, g0, p0, pn):
                    xgc = xgc_tiles[gi]
                    ps_c = psC.tile([P, pn * P], dt.float32, name="cps", tag="cps")
                    for bl in range(pn):
                        j = p0 + bl
                        blk = g0 + j
                        basec = sum(int(cc_ct[g0 + jj]) for jj in range(j))
                        n_mmc = int(cc_ct[blk])
                        for c in range(n_mmc):
                            oh_t = pa.tile([P, P], dt.bfloat16, name="ohc", tag="ohc")
                            col = ctp_ctr[0]
                            nc.vector.tensor_scalar(
                                out=oh_t[:], in0=sb_iotan[:],
                                scalar1=sb_dlsnc[:, col:col + 1],
                                scalar2=sb_invc[:, col:col + 1],
                                op0=OP.is_equal, op1=OP.mult)
                            ctp_ctr[0] += 1
                            nc.tensor.matmul(
                                ps_c[:, bl * P:(bl + 1) * P],
                                lhsT=xgc[:, basec + c, :], rhs=oh_t[:],
                                start=(c == 0), stop=(c == n_mmc - 1))
                    blk0 = g0 + p0
                    nc.scalar.activation(aggC[:, blk0 * P:(blk0 + pn) * P],
                                         ps_c[:], AF.Copy)

                ct_it = iter(ct_units())
                for i, c0 in enumerate(range(0, cfg.nt_pad, CHUNK)):
                    emit_chunk(t_xT, sb_wpt, xtn, c0,
                               min(CHUNK, cfg.nt_pad - c0), True)
                    if i >= 2:
                        u = next(ct_it, None)
                        if u is not None:
                            emit_ct_unit(*u)
                for u in ct_it:
                    emit_ct_unit(*u)

            # ======================= Phase B: tt + epilogue =======================
            piece_ctr = [0]
            oh_tick = [0]

            with tc.tile_pool(name="pb", bufs=2) as pb, \
                 tc.tile_pool(name="pxg", bufs=3) as pxg, \
                 tc.tile_pool(name="oh", bufs=24) as poh, \
                 tc.tile_pool(name="ohsq", bufs=4) as psq, \
                 tc.tile_pool(name="psAgg", bufs=2, space="PSUM") as psG, \
                 tc.tile_pool(name="psMid", bufs=2, space="PSUM") as psM, \
                 tc.tile_pool(name="psOut", bufs=2, space="PSUM") as psO:

                def make_oh_tt():
                    col = piece_ctr[0]
                    piece_ctr[0] += 1
                    oh_t = poh.tile([P, P], dt.bfloat16, name="oh", tag="oh")
                    if oh_tick[0] % cfg.act_oh_every == cfg.act_oh_every - 1:
                        sq = psq.tile([P, P], dt.bfloat16, name="sq", tag="sq")
                        nc.scalar.activation(sq[:], sb_iota[:], AF.Square,
                                             bias=sb_dlsn[:, col:col + 1])
                        nc.scalar.activation(oh_t[:], sq[:], AF.Relu,
                                             scale=sb_invn[:, col:col + 1],
                                             bias=sb_inv[:, col:col + 1])
                    else:
                        nc.vector.tensor_scalar(
                            out=oh_t[:], in0=sb_iotan[:],
                            scalar1=sb_dlsn[:, col:col + 1],
                            scalar2=sb_inv[:, col:col + 1],
                            op0=OP.is_equal, op1=OP.mult)
                    oh_tick[0] += 1
                    return oh_t

                for gi, (g0, gn) in enumerate(groups):
                    xg = {}
                    for d in range(2):
                        for b in range(nbin):
                            L = tt_call_len[(gi, d, b)]
                            t = pxg.tile([P, L // P, P], dt.bfloat16,
                                         name=f"xg{d}{b}", tag=f"xg{d}{b}")
                            nc.gpsimd.dma_gather(
                                out_ap=t[:],
                                in_ap=xtn[cfg.bins[b]:cfg.bins[b + 1], :],
                                idxs_ap=sb_idx[b][:, tt_call_off[(gi, d, b)] // 16:
                                                  (tt_call_off[(gi, d, b)] + L) // 16],
                                num_idxs=L, num_idxs_reg=L,
                                elem_size=P, single_packet=False)
                            xg[(d, b)] = t

                    agg_sb = {}
                    for p0 in range(0, gn, 2):
                        pn = min(2, gn - p0)
                        ps_agg = psG.tile([P, pn * 256], dt.float32,
                                          name="agg", tag="agg")
                        for bl in range(pn):
                            j = p0 + bl
                            blk = g0 + j
                            for d in range(2):
                                k = 0
                                n_mm = int(sum(cc_tt[blk, b, d] for b in range(nbin)))
                                for b in range(nbin):
                                    base = sum(int(cc_tt[g0 + jj, b, d])
                                               for jj in range(j))
                                    for c in range(int(cc_tt[blk, b, d])):
                                        oh_t = make_oh_tt()
                                        nc.tensor.matmul(
                                            ps_agg[:, bl * 256 + d * P:
                                                   bl * 256 + (d + 1) * P],
                                            lhsT=xg[(d, b)][:, base + c, :],
                                            rhs=oh_t[:],
                                            start=(k == 0), stop=(k == n_mm - 1))
                                        k += 1
                        sb_a = pb.tile([P, pn * 256], dt.bfloat16, name="aggsb",
                                       tag="aggsb")
                        nc.scalar.activation(sb_a[:], ps_agg[:], AF.Copy)
                        for bl in range(pn):
                            agg_sb[p0 + bl] = (sb_a, bl * 256)

                    og = pb.tile([P, gn * P], dt.bfloat16, name="og", tag="og")
                    for h0 in range(0, gn, 4):
                        hn = min(4, gn - h0)
                        ps_mid = psM.tile([P, hn * P], dt.float32, name="mid",
                                          tag="mid")
                        for bl in range(hn):
                            j = h0 + bl
                            blk = g0 + j
                            sb_a, off = agg_sb[j]
                            reg = ps_mid[:, bl * P:bl * P + P]
                            nc.tensor.matmul(reg, lhsT=sb_w1[:],
                                             rhs=xt_mine[:, P * blk:P * blk + P],
                                             start=True, stop=False)
                            nc.tensor.matmul(reg, lhsT=sb_ws[:],
                                             rhs=sb_a[:, off:off + P],
                                             start=False, stop=False)
                            nc.tensor.matmul(reg, lhsT=sb_wd[:],
                                             rhs=sb_a[:, off + P:off + 2 * P],
                                             start=False, stop=False)
                            nc.tensor.matmul(reg, lhsT=sb_wc[:],
                                             rhs=aggC[:, P * blk:P * blk + P],
                                             start=False, stop=True)
                        sb_mid = pb.tile([P, hn * P], dt.bfloat16, name="mid_sb",
                                         tag="mid_sb")
                        nc.scalar.activation(sb_mid[:], ps_mid[:], AF.Relu,
                                             bias=sb_bmid[:, 0:1])
                        ps_out = psO.tile([P, hn * P], dt.float32, name="out_ps",
                                          tag="out_ps")
                        for bl in range(hn):
                            nc.tensor.matmul(ps_out[:, bl * P:bl * P + P],
                                             lhsT=sb_wo[:],
                                             rhs=sb_mid[:, bl * P:bl * P + P],
                                             start=True, stop=True)
                        nc.scalar.activation(og[:, h0 * P:(h0 + hn) * P], ps_out[:],
                                             AF.Identity, bias=sb_bout[:, 0:1])
                    nc.sync.dma_start(t_out.ap()[:, P * g0:P * (g0 + gn)],
                                      og[:, :P * gn])

    nc.compile()
    return nc


def preprocess(inputs, cfg: Cfg):
    xt = np.asarray(inputs["x_target"], F32)
    xc = np.asarray(inputs["x_context"], F32)
    ett = np.asarray(inputs["edge_tt"]).astype(np.int64)
    ecs = np.asarray(inputs["edge_ct_src"]).astype(np.int64)
    ecd = np.asarray(inputs["edge_ct_dst"]).astype(np.int64)
    nblk, nbin = cfg.nblk, cfg.nbin
    groups = _groups(cfg)

    def fold_bias(x, W, b):
        if not np.any(b):
            return x
        c = np.linalg.lstsq(np.asarray(W, np.float64).T,
                            np.asarray(b, np.float64), rcond=None)[0]
        return x + c[None, :].astype(F32)

    Wp_t = np.asarray(inputs["Wp_t"], F32)
    Wp_c = np.asarray(inputs["Wp_c"], F32)
    bp_t = np.asarray(inputs["bp_t"], F32)
    bp_c = np.asarray(inputs["bp_c"], F32)
    xtf = fold_bias(xt, Wp_t, bp_t)
    xcf = fold_bias(xc, Wp_c, bp_c)

    xtT = np.zeros((P, cfg.nt_pad), BF16)
    xtT[:, :xt.shape[0]] = xtf.T.astype(BF16)
    xcT = np.zeros((P, cfg.nc_pad), BF16)
    xcT[:, :xc.shape[0]] = xcf.T.astype(BF16)

    W_self = np.asarray(inputs["W_self"], F32)
    W_ct_r = np.asarray(inputs["W_ct_r"], F32)
    w1 = 0.5 * W_self + 0.5 * W_ct_r + np.eye(P, dtype=F32)
    ws = 0.25 * np.asarray(inputs["W_s2d"], F32)
    wd = 0.25 * np.asarray(inputs["W_d2s"], F32)
    wc = 0.5 * np.asarray(inputs["W_ct_l"], F32)
    wo = np.asarray(inputs["W_out"], F32)
    bmid = (0.5 * np.asarray(inputs["b_self"], F32)
            + 0.25 * np.asarray(inputs["b_s2d"], F32)
            + 0.25 * np.asarray(inputs["b_d2s"], F32)
            + 0.5 * np.asarray(inputs["b_ct_l"], F32))
    bout = np.asarray(inputs["b_out"], F32)

    iota = np.arange(P, dtype=F32)
    shared = {
        "xcT": xcT,
        "wpt": np.ascontiguousarray(Wp_t.astype(BF16)),
        "wpc": np.ascontiguousarray(Wp_c.astype(BF16)),
        "bpt": bp_t.reshape(P, 1),
        "w1": w1.astype(BF16), "ws": ws.astype(BF16), "wd": wd.astype(BF16),
        "wc": wc.astype(BF16), "wo": wo.astype(BF16),
        "bmid": bmid.reshape(P, 1), "bout": bout.reshape(P, 1),
        "iota": np.ascontiguousarray(np.broadcast_to(iota, (P, P)).astype(BF16)),
        "iotan": np.ascontiguousarray(np.broadcast_to(-iota, (P, P)).astype(BF16)),
    }

    # per-core rotated source locations
    dirs = [
        ("s", ett[1], ett[0], True),
        ("d", ett[0], ett[1], True),
        ("c", ecd, ecs, False),
    ]

    pre = {}
    for nm, key, gnode, is_tt in dirs:
        core = (key // cfg.shard).astype(np.int64)
        blk = ((key % cfg.shard) // P).astype(np.int64)
        dloc = (key % P).astype(F32)
        cnt = np.bincount(key, minlength=cfg.nt_pad)
        inv = (1.0 / np.maximum(cnt, 1)).astype(F32)
        invv = inv[key]
        if is_tt:
            rot = (gnode - core * cfg.shard) % cfg.nt_pad
            r2 = _perm_rows(rot)
            bins = np.asarray(cfg.bins)
            bin_ = np.searchsorted(bins, r2, side="right") - 1
            loc = (r2 - bins[bin_]).astype(np.int16)
            cell = ((core * nblk + blk) * nbin + bin_)
            ncell = NCORE * nblk * nbin
        else:
            loc = _perm_rows(gnode).astype(np.int16)
            cell = core * nblk + blk
            ncell = NCORE * nblk
        order = np.argsort(cell, kind="stable")
        counts = np.bincount(cell, minlength=ncell)
        pre[nm] = dict(order=order, cell_s=cell[order], counts=counts,
                       loc=loc, dloc=dloc, invv=invv)

    cnt_s = pre["s"]["counts"].reshape(NCORE, nblk, nbin)
    cnt_d = pre["d"]["counts"].reshape(NCORE, nblk, nbin)
    cnt_c = pre["c"]["counts"].reshape(NCORE, nblk)
    cc_tt = np.zeros((nblk, nbin, 2), np.int64)
    cc_tt[:, :, 0] = np.maximum(-(-cnt_s.max(axis=0) // P), 1)
    cc_tt[:, :, 1] = np.maximum(-(-cnt_d.max(axis=0) // P), 1)
    cc_ct = np.maximum(-(-cnt_c.max(axis=0) // P), 1)

    def blk_offs(cc):
        if cc.ndim == 2:
            offs = np.zeros((nblk, nbin), np.int64)
            for b in range(nbin):
                o = 0
                for gi, (g0, gn) in enumerate(groups):
                    for j in range(gn):
                        offs[g0 + j, b] = o
                        o += int(cc[g0 + j, b]) * P
            return offs
        offs = np.zeros(nblk, np.int64)
        o = 0
        for gi, (g0, gn) in enumerate(groups):
            for j in range(gn):
                offs[g0 + j] = o
                o += int(cc[g0 + j]) * P
        return offs

    offs_s = blk_offs(cc_tt[:, :, 0])
    offs_d = blk_offs(cc_tt[:, :, 1])
    offs_c = blk_offs(cc_ct)

    def fill(nm, cc, offs):
        d = pre[nm]
        order, cell_s, counts = d["order"], d["cell_s"], d["counts"]
        starts = np.concatenate([[0], np.cumsum(counts)[:-1]])
        loc_s = d["loc"][order]
        dloc_s = d["dloc"][order]
        invv_s = d["invv"][order]
        is_tt = cc.ndim == 2
        cap = cc * P
        if is_tt:
            bin_len = [int(cap[:, b].sum()) for b in range(nbin)]
        else:
            bin_len = [int(cap.sum())]
        pos_in_cell = np.arange(len(cell_s)) - starts[cell_s]
        if is_tt:
            core_of = cell_s // (nblk * nbin)
            blk_of = (cell_s // nbin) % nblk
            bin_of = cell_s % nbin
            binbase = np.concatenate([[0], np.cumsum(bin_len)])[:-1]
            gslot = binbase[bin_of] + offs[blk_of, bin_of] + pos_in_cell
        else:
            core_of = cell_s // nblk
            blk_of = cell_s % nblk
            gslot = offs[blk_of] + pos_in_cell
        total = int(sum(bin_len))
        idx = np.zeros((NCORE, total), np.int16)
        dls = np.full((NCORE, total), -1.0, F32)
        inv = np.zeros((NCORE, total), F32)
        idx[core_of, gslot] = loc_s
        dls[core_of, gslot] = dloc_s
        inv[core_of, gslot] = invv_s
        if is_tt:
            binbase = np.concatenate([[0], np.cumsum(bin_len)])
            return ([idx[:, binbase[b]:binbase[b + 1]] for b in range(nbin)],
                    [dls[:, binbase[b]:binbase[b + 1]] for b in range(nbin)],
                    [inv[:, binbase[b]:binbase[b + 1]] for b in range(nbin)])
        return [idx], [dls], [inv]

    idx_s, dls_s, inv_s = fill("s", cc_tt[:, :, 0], offs_s)
    idx_d, dls_d, inv_d = fill("d", cc_tt[:, :, 1], offs_d)
    idx_c, dls_c, inv_c = fill("c", cc_ct, offs_c)

    def tt_piece_stream(core):
        cols_d, cols_i = [], []
        for gi, (g0, gn) in enumerate(groups):
            for j in range(gn):
                blk = g0 + j
                for d in range(2):
                    dls_bins = dls_s if d == 0 else dls_d
                    inv_bins = inv_s if d == 0 else inv_d
                    offs = offs_s if d == 0 else offs_d
                    ccd = cc_tt[:, :, d]
                    for b in range(nbin):
                        o = int(offs[blk, b])
                        for c in range(int(ccd[blk, b])):
                            cols_d.append(dls_bins[b][core, o + c * P:o + (c + 1) * P])
                            cols_i.append(inv_bins[b][core, o + c * P:o + (c + 1) * P])
        return (np.ascontiguousarray(np.stack(cols_d, axis=1)),
                np.ascontiguousarray(np.stack(cols_i, axis=1)))

    def ct_piece_stream(core):
        cols_d, cols_i = [], []
        for gi, (g0, gn) in enumerate(groups):
            for j in range(gn):
                blk = g0 + j
                o = int(offs_c[blk])
                for c in range(int(cc_ct[blk])):
                    cols_d.append(dls_c[0][core, o + c * P:o + (c + 1) * P])
                    cols_i.append(inv_c[0][core, o + c * P:o + (c + 1) * P])
        return (np.ascontiguousarray(np.stack(cols_d, axis=1)),
                np.ascontiguousarray(np.stack(cols_i, axis=1)))

    in_maps = []
    for k in range(NCORE):
        m = dict(shared)
        m["xT"] = np.roll(xtT, -cfg.shard * k, axis=1)
        for b in range(nbin):
            segs = []
            for gi, (g0, gn) in enumerate(groups):
                for d in range(2):
                    src = idx_s[b] if d == 0 else idx_d[b]
                    offs = offs_s if d == 0 else offs_d
                    o = int(offs[g0, b])
                    L = int(sum(cc_tt[g0 + j, b, d] for j in range(gn))) * P
                    segs.append(src[k, o:o + L])
            m[f"idx{b}"] = _wrap_idx(np.concatenate(segs))
        m["idxc"] = _wrap_idx(idx_c[0][k])
        dls_t, inv_t = tt_piece_stream(k)
        m["dlsn"] = np.ascontiguousarray(-dls_t)
        m["invv"] = inv_t
        m["invvn"] = np.ascontiguousarray(-inv_t)
        dls_ct, inv_ct = ct_piece_stream(k)
        m["dlsnc"] = np.ascontiguousarray(-dls_ct)
        m["invvc"] = inv_ct
        in_maps.append(m)
    return in_maps, cc_tt, cc_ct


def run(inputs, cfg: Cfg, trace=False):
    in_maps, cc_tt, cc_ct = preprocess(inputs, cfg)
    key = (cfg, cc_tt.tobytes(), cc_ct.tobytes())
    if key not in _prog_cache:
        _prog_cache[key] = build_program(cfg, cc_tt, cc_ct)
    nc = _prog_cache[key]
    res = bass_utils.run_bass_kernel_spmd(nc, in_maps, core_ids=list(range(NCORE)),
                                          trace=trace)
    outT = np.concatenate([res.results[k]["outT"] for k in range(NCORE)], axis=1)
    n_t = np.asarray(inputs["x_target"]).shape[0]
    out = outT[:, :n_t].T.astype(F32)
    return out, res


def kernel(**inputs) -> np.ndarray:
    out, _ = run(inputs, FULL, trace=False)
    return out


# revision 50
# speedup vs baseline: 1.1705x; 1.0044x over previous
"""Trainium2 Bass kernel for nn_HeteroForecastSageConv (v4).

Strategy (8 NeuronCores, SPMD, dst-sharded):
 - Each core owns 12800 target nodes. Inputs are host-rotated per core
   so the own shard is always columns [0:12800) of xT; the
   pretransformed tables (x_t, x_c) are computed replicated with the
   stationary-operand matmul trick (lhsT = x^T tile, rhs = W) yielding
   node-major PSUM directly (no transposes). Per-type bias is folded
   into the input on the host (x' = x + b @ W^-1; zero here).
 - Phase A window also runs the whole ct pipeline: Pool does ct
   gathers, DVE builds ct one-hots between relu work, PE accumulates
   aggC into persistent SBUF. xt_mine (feature-major own shard) is
   computed from the same streamed chunks.
 - Phase B: tt gathers (Pool) + fused one-hot + matmul aggregation
   feature-major, then the folded epilogue. The one-hot mean-scale is
   fused: DVE tensor_scalar (-iota == -dloc) * inv. Table rows are
   chunk-permuted so phase-A writes are 4KB-contiguous per partition.
 - Engine split: x reads + most table writes + output on SP; 1/3
   writes + some reads on Act; relu 5/9 DVE 4/9 Act; gathers + idx
   stream loads on Pool only.

Math (alpha = 0.5, folded on host):
  x_mid = x_t @ (0.5 W_self + 0.5 W_ct_r + I) + aggS @ (0.25 W_s2d)
        + aggD @ (0.25 W_d2s) + aggC @ (0.5 W_ct_l) + b_mid
  out   = relu(x_mid) @ W_out + b_out
"""
import sys
import dataclasses

sys.path.insert(0, "/opt/trn_rl_repo")

import numpy as np
import ml_dtypes

import concourse.bass as bass
import concourse.bacc as bacc
import concourse.mybir as mybir
import concourse.tile as tile
from concourse import bass_utils

BF16 = ml_dtypes.bfloat16
F32 = np.float32
NCORE = 8
P = 128


@dataclasses.dataclass(frozen=True)
class Cfg:
    n_t: int
    n_c: int
    shard: int
    nc_pad: int
    nbin: int
    grp: int
    act_oh_every: int = 10 ** 9   # every Nth tt one-hot goes to the Act engine

    @property
    def nt_pad(self):
        return self.shard * NCORE

    @property
    def nblk(self):
        return self.shard // P

    @property
    def bins(self):
        # 2048-aligned uneven bins, each <= 32767 for int16 gather indices
        assert self.nbin == 4 and self.nt_pad == 102400
        return [0, 26624, 53248, 79872, 102400]


FULL = Cfg(n_t=100000, n_c=20000, shard=12800, nc_pad=20480, nbin=4, grp=4)


def _perm_rows(r):
    """Table-row permutation: within each 2048-row chunk, node (128g + p)
    is stored at row (16p + g) so phase-A writes are 4KB-contiguous."""
    chunk = r // 2048
    o = r % 2048
    return chunk * 2048 + (o % P) * 16 + o // P

_prog_cache = {}


def _groups(cfg):
    # last two groups halved: shortens the serial phase-B drain pipeline
    gs = [(g0, min(cfg.grp, cfg.nblk - g0)) for g0 in range(0, cfg.nblk - 4, cfg.grp)]
    gs += [(cfg.nblk - 4, 2), (cfg.nblk - 2, 2)]
    return gs


def _wrap_idx(stream):
    assert stream.size % 16 == 0
    idx16 = stream.reshape(-1, 16).T
    return np.ascontiguousarray(np.tile(idx16, (8, 1)).astype(np.int16))


def build_program(cfg: Cfg, cc_tt, cc_ct):
    """cc_tt: [nblk, nbin, 2] tiles per tt cell; cc_ct: [nblk] (shared cores)."""
    dt = mybir.dt
    AF = mybir.ActivationFunctionType
    OP = mybir.AluOpType
    nblk, nbin = cfg.nblk, cfg.nbin
    groups = _groups(cfg)

    tt_bin_len = [0] * nbin
    tt_call_off = {}
    tt_call_len = {}
    for gi, (g0, gn) in enumerate(groups):
        for d in range(2):
            for b in range(nbin):
                tt_call_off[(gi, d, b)] = tt_bin_len[b]
                L = int(sum(cc_tt[g0 + j, b, d] for j in range(gn))) * P
                tt_call_len[(gi, d, b)] = L
                tt_bin_len[b] += L
    ct_len = 0
    ct_call_off = {}
    ct_call_len = {}
    for gi, (g0, gn) in enumerate(groups):
        ct_call_off[gi] = ct_len
        L = int(sum(cc_ct[g0 + j] for j in range(gn))) * P
        ct_call_len[gi] = L
        ct_len += L
    tt_pieces = int(cc_tt.sum())
    ct_pieces = int(cc_ct.sum())

    nc = bacc.Bacc("TRN2", target_bir_lowering=False, debug=False)

    def din(name, shape, d):
        return nc.dram_tensor(name, shape, d, kind="ExternalInput")

    t_xT = din("xT", [P, cfg.nt_pad], dt.bfloat16)
    t_xcT = din("xcT", [P, cfg.nc_pad], dt.bfloat16)
    t_wpt = din("wpt", [P, P], dt.bfloat16)
    t_wpc = din("wpc", [P, P], dt.bfloat16)
    t_bpt = din("bpt", [P, 1], dt.float32)
    t_w1 = din("w1", [P, P], dt.bfloat16)
    t_ws = din("ws", [P, P], dt.bfloat16)
    t_wd = din("wd", [P, P], dt.bfloat16)
    t_wc = din("wc", [P, P], dt.bfloat16)
    t_wo = din("wo", [P, P], dt.bfloat16)
    t_bmid = din("bmid", [P, 1], dt.float32)
    t_bout = din("bout", [P, 1], dt.float32)
    t_iota = din("iota", [P, P], dt.bfloat16)       # +i, for Act Square path
    t_iotan = din("iotan", [P, P], dt.bfloat16)     # -i, for DVE is_equal path
    t_idx = [din(f"idx{b}", [P, max(tt_bin_len[b], 16) // 16], dt.int16)
             for b in range(nbin)]
    t_idxc = din("idxc", [P, max(ct_len, 16) // 16], dt.int16)
    t_dlsn = din("dlsn", [P, tt_pieces], dt.float32)    # -dloc (pad +1)
    t_inv = din("invv", [P, tt_pieces], dt.float32)     # +inv
    t_invn = din("invvn", [P, tt_pieces], dt.float32)   # -inv
    t_dlsnc = din("dlsnc", [P, ct_pieces], dt.float32)
    t_invc = din("invvc", [P, ct_pieces], dt.float32)
    t_out = nc.dram_tensor("outT", [P, cfg.shard], dt.bfloat16, kind="ExternalOutput")

    with tile.TileContext(nc) as tc:
        with tc.tile_pool(name="dram", bufs=1, space="DRAM") as dpool, \
             tc.tile_pool(name="persist", bufs=1) as pp:
            xtn = dpool.tile([cfg.nt_pad, P], dt.bfloat16)
            xcn = dpool.tile([cfg.nc_pad, P], dt.bfloat16)

            def load(t, shape, d, eng=None):
                s = pp.tile(shape, d, name=f"sb_{t.name}")
                (eng or nc.sync).dma_start(s[:], t.ap())
                return s

            sb_wpt = load(t_wpt, [P, P], dt.bfloat16)
            sb_wpc = load(t_wpc, [P, P], dt.bfloat16)
            sb_bpt = load(t_bpt, [P, 1], dt.float32)
            sb_w1 = load(t_w1, [P, P], dt.bfloat16)
            sb_ws = load(t_ws, [P, P], dt.bfloat16)
            sb_wd = load(t_wd, [P, P], dt.bfloat16)
            sb_wc = load(t_wc, [P, P], dt.bfloat16)
            sb_wo = load(t_wo, [P, P], dt.bfloat16)
            sb_bmid = load(t_bmid, [P, 1], dt.float32)
            sb_bout = load(t_bout, [P, 1], dt.float32)
            sb_iota = load(t_iota, [P, P], dt.bfloat16)
            sb_iotan = load(t_iotan, [P, P], dt.bfloat16)
            # ct streams on Pool (idle at start); tt streams on SP
            sb_idxc = load(t_idxc, [P, max(ct_len, 16) // 16], dt.int16,
                           eng=nc.gpsimd)
            sb_dlsnc = load(t_dlsnc, [P, ct_pieces], dt.float32, eng=nc.gpsimd)
            sb_invc = load(t_invc, [P, ct_pieces], dt.float32, eng=nc.gpsimd)
            sb_idx = [load(t_idx[b], [P, max(tt_bin_len[b], 16) // 16],
                           dt.int16, eng=nc.gpsimd) for b in range(nbin)]
            sb_dlsn = load(t_dlsn, [P, tt_pieces], dt.float32, eng=nc.gpsimd)
            sb_inv = load(t_inv, [P, tt_pieces], dt.float32, eng=nc.gpsimd)
            sb_invn = load(t_invn, [P, tt_pieces], dt.float32, eng=nc.gpsimd)
            xt_mine = pp.tile([P, cfg.shard], dt.bfloat16)
            aggC = pp.tile([P, cfg.shard], dt.bfloat16)

            # =============== Phase A + ct pipeline (interleaved) ===============
            CHUNK = 2048
            relu_tick = [0]
            ctp_ctr = [0]
            wr_tick = [0]

            with tc.tile_pool(name="pa", bufs=4) as pa, \
                 tc.tile_pool(name="pgc", bufs=4) as pgc, \
                 tc.tile_pool(name="psA", bufs=4, space="PSUM") as psA, \
                 tc.tile_pool(name="psAm", bufs=2, space="PSUM") as psAm, \
                 tc.tile_pool(name="psC", bufs=2, space="PSUM") as psC:

                def emit_chunk(src_dram, w_sb, nodes_dram, c0, cw, mine):
                    sb_in = pa.tile([P, CHUNK], dt.bfloat16, name="a_in", tag="a_in")
                    r_eng = nc.scalar if (c0 // CHUNK) % 4 == 3 else nc.sync
                    r_eng.dma_start(sb_in[:, :cw], src_dram.ap()[:, c0:c0 + cw])
                    sb_nodes = pa.tile([P, CHUNK // P, P], dt.bfloat16,
                                       name="a_nodes", tag="a_nodes")
                    for s0 in range(0, cw, 512):
                        ps = psA.tile([P, 4, P], dt.float32, name="a_ps", tag="a_ps")
                        for j in range(4):
                            nc.tensor.matmul(ps[:, j, :],
                                             lhsT=sb_in[:, s0 + P * j:s0 + P * (j + 1)],
                                             rhs=w_sb[:], start=True, stop=True)
                        dst = sb_nodes[:, s0 // P:s0 // P + 4, :]
                        if relu_tick[0] % 9 < 5:
                            nc.vector.tensor_scalar(
                                out=dst, in0=ps[:], scalar1=0.0, scalar2=None,
                                op0=OP.max)
                        else:
                            nc.scalar.activation(dst, ps[:], AF.Relu)
                        relu_tick[0] += 1
                        if mine and c0 + s0 < cfg.shard:
                            psm = psAm.tile([P, 512], dt.float32, name="m_ps",
                                            tag="m_ps")
                            nc.tensor.matmul(psm[:], lhsT=w_sb[:],
                                             rhs=sb_in[:, s0:s0 + 512],
                                             start=True, stop=True)
                            if relu_tick[0] % 9 < 5:
                                nc.vector.tensor_scalar(
                                    out=xt_mine[:, c0 + s0:c0 + s0 + 512],
                                    in0=psm[:], scalar1=sb_bpt[:, 0:1], scalar2=0.0,
                                    op0=OP.add, op1=OP.max)
                            else:
                                nc.scalar.activation(
                                    xt_mine[:, c0 + s0:c0 + s0 + 512], psm[:],
                                    AF.Relu, bias=sb_bpt[:, 0:1])
                            relu_tick[0] += 1
                    w_eng = (nc.sync, nc.scalar, nc.sync)[wr_tick[0] % 3]
                    wr_tick[0] += 1
                    w_eng.dma_start(
                        nodes_dram[c0:c0 + cw, :].rearrange("(p g) f -> p g f", p=P),
                        sb_nodes[:, :cw // P, :])

                for c0 in range(0, cfg.nc_pad, CHUNK):
                    emit_chunk(t_xcT, sb_wpc, xcn, c0,
                               min(CHUNK, cfg.nc_pad - c0), False)

                # ct gathers, one per group (throttled by pgc pool)
                xgc_tiles = []
                for gi, (g0, gn) in enumerate(groups):
                    Lc = ct_call_len[gi]
                    t = pgc.tile([P, Lc // P, P], dt.bfloat16, name="xgc", tag="xgc")
                    nc.gpsimd.dma_gather(
                        out_ap=t[:], in_ap=xcn[:],
                        idxs_ap=sb_idxc[:, ct_call_off[gi] // 16:
                                        (ct_call_off[gi] + Lc) // 16],
                        num_idxs=Lc, num_idxs_reg=Lc,
                        elem_size=P, single_packet=False)
                    xgc_tiles.append(t)

                def ct_units():
                    for gi, (g0, gn) in enumerate(groups):
                        for p0 in range(0, gn, 2):
                            yield (gi, g0, p0, min(2, gn - p0))

                def emit_ct_unit(gi, g0, p0, pn):
                    xgc = xgc_tiles[gi]
                    ps_c = psC.tile([P, pn * P], dt.float32, name="cps", tag="cps")
                    for bl in range(pn):
                        j = p0 + bl
                        blk = g0 + j
                        basec = sum(int(cc_ct[g0 + jj]) for jj in range(j))
                        n_mmc = int(cc_ct[blk])
                        for c in range(n_mmc):
                            oh_t = pa.tile([P, P], dt.bfloat16, name="ohc", tag="ohc")
                            col = ctp_ctr[0]
                            nc.vector.tensor_scalar(
                                out=oh_t[:], in0=sb_iotan[:],
                                scalar1=sb_dlsnc[:, col:col + 1],
                                scalar2=sb_invc[:, col:col + 1],
                                op0=OP.is_equal, op1=OP.mult)
                            ctp_ctr[0] += 1
                            nc.tensor.matmul(
                                ps_c[:, bl * P:(bl + 1) * P],
                                lhsT=xgc[:, basec + c, :], rhs=oh_t[:],
                                start=(c == 0), stop=(c == n_mmc - 1))
                    blk0 = g0 + p0
                    nc.scalar.activation(aggC[:, blk0 * P:(blk0 + pn) * P],
                                         ps_c[:], AF.Copy)

                ct_it = iter(ct_units())
                for i, c0 in enumerate(range(0, cfg.nt_pad, CHUNK)):
                    emit_chunk(t_xT, sb_wpt, xtn, c0,
                               min(CHUNK, cfg.nt_pad - c0), True)
                    if i >= 2:
                        u = next(ct_it, None)
                        if u is not None:
                            emit_ct_unit(*u)
                for u in ct_it:
                    emit_ct_unit(*u)

            # ======================= Phase B: tt + epilogue =======================
            piece_ctr = [0]
            oh_tick = [0]

            with tc.tile_pool(name="pb", bufs=2) as pb, \
                 tc.tile_pool(name="pxg", bufs=3) as pxg, \
                 tc.tile_pool(name="oh", bufs=24) as poh, \
                 tc.tile_pool(name="ohsq", bufs=4) as psq, \
                 tc.tile_pool(name="psAgg", bufs=2, space="PSUM") as psG, \
                 tc.tile_pool(name="psMid", bufs=2, space="PSUM") as psM, \
                 tc.tile_pool(name="psOut", bufs=2, space="PSUM") as psO:

                def make_oh_tt():
                    col = piece_ctr[0]
                    piece_ctr[0] += 1
                    oh_t = poh.tile([P, P], dt.bfloat16, name="oh", tag="oh")
                    if oh_tick[0] % cfg.act_oh_every == cfg.act_oh_every - 1:
                        sq = psq.tile([P, P], dt.bfloat16, name="sq", tag="sq")
                        nc.scalar.activation(sq[:], sb_iota[:], AF.Square,
                                             bias=sb_dlsn[:, col:col + 1])
                        nc.scalar.activation(oh_t[:], sq[:], AF.Relu,
                                             scale=sb_invn[:, col:col + 1],
                                             bias=sb_inv[:, col:col + 1])
                    else:
                        nc.vector.tensor_scalar(
                            out=oh_t[:], in0=sb_iotan[:],
                            scalar1=sb_dlsn[:, col:col + 1],
                            scalar2=sb_inv[:, col:col + 1],
                            op0=OP.is_equal, op1=OP.mult)
                    oh_tick[0] += 1
                    return oh_t

                for gi, (g0, gn) in enumerate(groups):
                    xg = {}
                    for d in range(2):
                        for b in range(nbin):
                            L = tt_call_len[(gi, d, b)]
                            t = pxg.tile([P, L // P, P], dt.bfloat16,
                                         name=f"xg{d}{b}", tag=f"xg{d}{b}")
                            nc.gpsimd.dma_gather(
                                out_ap=t[:],
                                in_ap=xtn[cfg.bins[b]:cfg.bins[b + 1], :],
                                idxs_ap=sb_idx[b][:, tt_call_off[(gi, d, b)] // 16:
                                                  (tt_call_off[(gi, d, b)] + L) // 16],
                                num_idxs=L, num_idxs_reg=L,
                                elem_size=P, single_packet=False)
                            xg[(d, b)] = t

                    agg_sb = {}
                    for p0 in range(0, gn, 2):
                        pn = min(2, gn - p0)
                        ps_agg = psG.tile([P, pn * 256], dt.float32,
                                          name="agg", tag="agg")
                        for bl in range(pn):
                            j = p0 + bl
                            blk = g0 + j
                            for d in range(2):
                                k = 0
                                n_mm = int(sum(cc_tt[blk, b, d] for b in range(nbin)))
                                for b in range(nbin):
                                    base = sum(int(cc_tt[g0 + jj, b, d])
                                               for jj in range(j))
                                    for c in range(int(cc_tt[blk, b, d])):
                                        oh_t = make_oh_tt()
                                        nc.tensor.matmul(
                                            ps_agg[:, bl * 256 + d * P:
                                                   bl * 256 + (d + 1) * P],
                                            lhsT=xg[(d, b)][:, base + c, :],
                                            rhs=oh_t[:],
                                            start=(k == 0), stop=(k == n_mm - 1))
                                        k += 1
                        sb_a = pb.tile([P, pn * 256], dt.bfloat16, name="aggsb",
                                       tag="aggsb")
                        nc.scalar.activation(sb_a[:], ps_agg[:], AF.Copy)
                        for bl in range(pn):
                            agg_sb[p0 + bl] = (sb_a, bl * 256)

                    og = pb.tile([P, gn * P], dt.bfloat16, name="og", tag="og")
                    for h0 in range(0, gn, 4):
                        hn = min(4, gn - h0)
                        ps_mid = psM.tile([P, hn * P], dt.float32, name="mid",
                                          tag="mid")
                        for bl in range(hn):
                            j = h0 + bl
                            blk = g0 + j
                            sb_a, off = agg_sb[j]
                            reg = ps_mid[:, bl * P:bl * P + P]
                            nc.tensor.matmul(reg, lhsT=sb_w1[:],
                                             rhs=xt_mine[:, P * blk:P * blk + P],
                                             start=True, stop=False)
                            nc.tensor.matmul(reg, lhsT=sb_ws[:],
                                             rhs=sb_a[:, off:off + P],
                                             start=False, stop=False)
                            nc.tensor.matmul(reg, lhsT=sb_wd[:],
                                             rhs=sb_a[:, off + P:off + 2 * P],
                                             start=False, stop=False)
                            nc.tensor.matmul(reg, lhsT=sb_wc[:],
                                             rhs=aggC[:, P * blk:P * blk + P],
                                             start=False, stop=True)
                        sb_mid = pb.tile([P, hn * P], dt.bfloat16, name="mid_sb",
                                         tag="mid_sb")
                        nc.scalar.activation(sb_mid[:], ps_mid[:], AF.Relu,
                                             bias=sb_bmid[:, 0:1])
                        ps_out = psO.tile([P, hn * P], dt.float32, name="out_ps",
                                          tag="out_ps")
                        for bl in range(hn):
                            nc.tensor.matmul(ps_out[:, bl * P:bl * P + P],
                                             lhsT=sb_wo[:],
                                             rhs=sb_mid[:, bl * P:bl * P + P],
                                             start=True, stop=True)
                        nc.scalar.activation(og[:, h0 * P:(h0 + hn) * P], ps_out[:],
                                             AF.Identity, bias=sb_bout[:, 0:1])
                    nc.sync.dma_start(t_out.ap()[:, P * g0:P * (g0 + gn)],
                                      og[:, :P * gn])

    nc.compile()
    return nc


def preprocess(inputs, cfg: Cfg):
    xt = np.asarray(inputs["x_target"], F32)
    xc = np.asarray(inputs["x_context"], F32)
    ett = np.asarray(inputs["edge_tt"]).astype(np.int64)
    ecs = np.asarray(inputs["edge_ct_src"]).astype(np.int64)
    ecd = np.asarray(inputs["edge_ct_dst"]).astype(np.int64)
    nblk, nbin = cfg.nblk, cfg.nbin
    groups = _groups(cfg)

    def fold_bias(x, W, b):
        if not np.any(b):
            return x
        c = np.linalg.lstsq(np.asarray(W, np.float64).T,
                            np.asarray(b, np.float64), rcond=None)[0]
        return x + c[None, :].astype(F32)

    Wp_t = np.asarray(inputs["Wp_t"], F32)
    Wp_c = np.asarray(inputs["Wp_c"], F32)
    bp_t = np.asarray(inputs["bp_t"], F32)
    bp_c = np.asarray(inputs["bp_c"], F32)
    xtf = fold_bias(xt, Wp_t, bp_t)
    xcf = fold_bias(xc, Wp_c, bp_c)

    xtT = np.zeros((P, cfg.nt_pad), BF16)
    xtT[:, :xt.shape[0]] = xtf.T.astype(BF16)
    xcT = np.zeros((P, cfg.nc_pad), BF16)
    xcT[:, :xc.shape[0]] = xcf.T.astype(BF16)

    W_self = np.asarray(inputs["W_self"], F32)
    W_ct_r = np.asarray(inputs["W_ct_r"], F32)
    w1 = 0.5 * W_self + 0.5 * W_ct_r + np.eye(P, dtype=F32)
    ws = 0.25 * np.asarray(inputs["W_s2d"], F32)
    wd = 0.25 * np.asarray(inputs["W_d2s"], F32)
    wc = 0.5 * np.asarray(inputs["W_ct_l"], F32)
    wo = np.asarray(inputs["W_out"], F32)
    bmid = (0.5 * np.asarray(inputs["b_self"], F32)
            + 0.25 * np.asarray(inputs["b_s2d"], F32)
            + 0.25 * np.asarray(inputs["b_d2s"], F32)
            + 0.5 * np.asarray(inputs["b_ct_l"], F32))
    bout = np.asarray(inputs["b_out"], F32)

    iota = np.arange(P, dtype=F32)
    shared = {
        "xcT": xcT,
        "wpt": np.ascontiguousarray(Wp_t.astype(BF16)),
        "wpc": np.ascontiguousarray(Wp_c.astype(BF16)),
        "bpt": bp_t.reshape(P, 1),
        "w1": w1.astype(BF16), "ws": ws.astype(BF16), "wd": wd.astype(BF16),
        "wc": wc.astype(BF16), "wo": wo.astype(BF16),
        "bmid": bmid.reshape(P, 1), "bout": bout.reshape(P, 1),
        "iota": np.ascontiguousarray(np.broadcast_to(iota, (P, P)).astype(BF16)),
        "iotan": np.ascontiguousarray(np.broadcast_to(-iota, (P, P)).astype(BF16)),
    }

    # per-core rotated source locations
    dirs = [
        ("s", ett[1], ett[0], True),
        ("d", ett[0], ett[1], True),
        ("c", ecd, ecs, False),
    ]

    pre = {}
    for nm, key, gnode, is_tt in dirs:
        core = (key // cfg.shard).astype(np.int64)
        blk = ((key % cfg.shard) // P).astype(np.int64)
        dloc = (key % P).astype(F32)
        cnt = np.bincount(key, minlength=cfg.nt_pad)
        inv = (1.0 / np.maximum(cnt, 1)).astype(F32)
        invv = inv[key]
        if is_tt:
            rot = (gnode - core * cfg.shard) % cfg.nt_pad
            r2 = _perm_rows(rot)
            bins = np.asarray(cfg.bins)
            bin_ = np.searchsorted(bins, r2, side="right") - 1
            loc = (r2 - bins[bin_]).astype(np.int16)
            cell = ((core * nblk + blk) * nbin + bin_)
            ncell = NCORE * nblk * nbin
        else:
            loc = _perm_rows(gnode).astype(np.int16)
            cell = core * nblk + blk
            ncell = NCORE * nblk
        order = np.argsort(cell, kind="stable")
        counts = np.bincount(cell, minlength=ncell)
        pre[nm] = dict(order=order, cell_s=cell[order], counts=counts,
                       loc=loc, dloc=dloc, invv=invv)

    cnt_s = pre["s"]["counts"].reshape(NCORE, nblk, nbin)
    cnt_d = pre["d"]["counts"].reshape(NCORE, nblk, nbin)
    cnt_c = pre["c"]["counts"].reshape(NCORE, nblk)
    cc_tt = np.zeros((nblk, nbin, 2), np.int64)
    cc_tt[:, :, 0] = np.maximum(-(-cnt_s.max(axis=0) // P), 1)
    cc_tt[:, :, 1] = np.maximum(-(-cnt_d.max(axis=0) // P), 1)
    cc_ct = np.maximum(-(-cnt_c.max(axis=0) // P), 1)

    def blk_offs(cc):
        if cc.ndim == 2:
            offs = np.zeros((nblk, nbin), np.int64)
            for b in range(nbin):
                o = 0
                for gi, (g0, gn) in enumerate(groups):
                    for j in range(gn):
                        offs[g0 + j, b] = o
                        o += int(cc[g0 + j, b]) * P
            return offs
        offs = np.zeros(nblk, np.int64)
        o = 0
        for gi, (g0, gn) in enumerate(groups):
            for j in range(gn):
                offs[g0 + j] = o
                o += int(cc[g0 + j]) * P
        return offs

    offs_s = blk_offs(cc_tt[:, :, 0])
    offs_d = blk_offs(cc_tt[:, :, 1])
    offs_c = blk_offs(cc_ct)

    def fill(nm, cc, offs):
        d = pre[nm]
        order, cell_s, counts = d["order"], d["cell_s"], d["counts"]
        starts = np.concatenate([[0], np.cumsum(counts)[:-1]])
        loc_s = d["loc"][order]
        dloc_s = d["dloc"][order]
        invv_s = d["invv"][order]
        is_tt = cc.ndim == 2
        cap = cc * P
        if is_tt:
            bin_len = [int(cap[:, b].sum()) for b in range(nbin)]
        else:
            bin_len = [int(cap.sum())]
        pos_in_cell = np.arange(len(cell_s)) - starts[cell_s]
        if is_tt:
            core_of = cell_s // (nblk * nbin)
            blk_of = (cell_s // nbin) % nblk
            bin_of = cell_s % nbin
            binbase = np.concatenate([[0], np.cumsum(bin_len)])[:-1]
            gslot = binbase[bin_of] + offs[blk_of, bin_of] + pos_in_cell
        else:
            core_of = cell_s // nblk
            blk_of = cell_s % nblk
            gslot = offs[blk_of] + pos_in_cell
        total = int(sum(bin_len))
        idx = np.zeros((NCORE, total), np.int16)
        dls = np.full((NCORE, total), -1.0, F32)
        inv = np.zeros((NCORE, total), F32)
        idx[core_of, gslot] = loc_s
        dls[core_of, gslot] = dloc_s
        inv[core_of, gslot] = invv_s
        if is_tt:
            binbase = np.concatenate([[0], np.cumsum(bin_len)])
            return ([idx[:, binbase[b]:binbase[b + 1]] for b in range(nbin)],
                    [dls[:, binbase[b]:binbase[b + 1]] for b in range(nbin)],
                    [inv[:, binbase[b]:binbase[b + 1]] for b in range(nbin)])
        return [idx], [dls], [inv]

    idx_s, dls_s, inv_s = fill("s", cc_tt[:, :, 0], offs_s)
    idx_d, dls_d, inv_d = fill("d", cc_tt[:, :, 1], offs_d)
    idx_c, dls_c, inv_c = fill("c", cc_ct, offs_c)

    def tt_piece_stream(core):
        cols_d, cols_i = [], []
        for gi, (g0, gn) in enumerate(groups):
            for j in range(gn):
                blk = g0 + j
                for d in range(2):
                    dls_bins = dls_s if d == 0 else dls_d
                    inv_bins = inv_s if d == 0 else inv_d
                    offs = offs_s if d == 0 else offs_d
                    ccd = cc_tt[:, :, d]
                    for b in range(nbin):
                        o = int(offs[blk, b])
                        for c in range(int(ccd[blk, b])):
                            cols_d.append(dls_bins[b][core, o + c * P:o + (c + 1) * P])
                            cols_i.append(inv_bins[b][core, o + c * P:o + (c + 1) * P])
        return (np.ascontiguousarray(np.stack(cols_d, axis=1)),
                np.ascontiguousarray(np.stack(cols_i, axis=1)))

    def ct_piece_stream(core):
        cols_d, cols_i = [], []
        for gi, (g0, gn) in enumerate(groups):
            for j in range(gn):
                blk = g0 + j
                o = int(offs_c[blk])
                for c in range(int(cc_ct[blk])):
                    cols_d.append(dls_c[0][core, o + c * P:o + (c + 1) * P])
                    cols_i.append(inv_c[0][core, o + c * P:o + (c + 1) * P])
        return (np.ascontiguousarray(np.stack(cols_d, axis=1)),
                np.ascontiguousarray(np.stack(cols_i, axis=1)))

    in_maps = []
    for k in range(NCORE):
        m = dict(shared)
        m["xT"] = np.roll(xtT, -cfg.shard * k, axis=1)
        for b in range(nbin):
            segs = []
            for gi, (g0, gn) in enumerate(groups):
                for d in range(2):
                    src = idx_s[b] if d == 0 else idx_d[b]
                    offs = offs_s if d == 0 else offs_d
                    o = int(offs[g0, b])
                    L = int(sum(cc_tt[g0 + j, b, d] for j in range(gn))) * P
                    segs.append(src[k, o:o + L])
            m[f"idx{b}"] = _wrap_idx(np.concatenate(segs))
        m["idxc"] = _wrap_idx(idx_c[0][k])
        dls_t, inv_t = tt_piece_stream(k)
        m["dlsn"] = np.ascontiguousarray(-dls_t)
        m["invv"] = inv_t
        m["invvn"] = np.ascontiguousarray(-inv_t)
        dls_ct, inv_ct = ct_piece_stream(k)
        m["dlsnc"] = np.ascontiguousarray(-dls_ct)
        m["invvc"] = inv_ct
        in_maps.append(m)
    return in_maps, cc_tt, cc_ct


def run(inputs, cfg: Cfg, trace=False):
    in_maps, cc_tt, cc_ct = preprocess(inputs, cfg)
    key = (cfg, cc_tt.tobytes(), cc_ct.tobytes())
    if key not in _prog_cache:
        _prog_cache[key] = build_program(cfg, cc_tt, cc_ct)
    nc = _prog_cache[key]
    res = bass_utils.run_bass_kernel_spmd(nc, in_maps, core_ids=list(range(NCORE)),
                                          trace=trace)
    outT = np.concatenate([res.results[k]["outT"] for k in range(NCORE)], axis=1)
    n_t = np.asarray(inputs["x_target"]).shape[0]
    out = outT[:, :n_t].T.astype(F32)
    return out, res


def kernel(**inputs) -> np.ndarray:
    out, _ = run(inputs, FULL, trace=False)
    return out
